# revision 40
# baseline (speedup 1.0000x reference)
"""Multi-head causal self-attention (B=2, S=2048, C=1024, H=16) on 8 TRN2
NeuronCores.

Sharding: data-parallel over batch x tensor-parallel over heads.  Core c
handles batch b = c//4 and the 4 heads g = c%4 -> heads [4g, 4g+4).  Each core
computes its QKV projections from the full x[b] (weights column-sharded
head-wise), runs causal attention for its 4 heads, and writes a [S, 256]
output shard.  No cross-device communication.

Numerics: mixed bf16/fp8.  fp8e4 (TRN E4M3, max 240) matmuls with
perf_mode=DoubleRow stream 2 contraction-tiles per pass (measured 216ns/MM at
N=512 - exactly 2x bf16 throughput, LDWEIGHTS fully hidden).  fp8 error
averages out over diffuse softmax rows (q >= 512 has >= 513-way softmax, max
weight ~1%), so everything fp8-able for those rows is fp8:
  - q/k projections for s-chunks 1-3: fp8-DR (inputs x8 = fp8(x^T),
    wq8/wk8 = fp8(8*W); the 8x weight scale keeps W ~N(0,0.02) out of e4m3's
    subnormal range; exp scale absorbs the 64x on scores).
  - v projection st>=4 (k >= 512): fp8 operands (bf16-rate non-DR matmuls),
    v stored fp8 with a ones column (PV row sums) padded to 68 cols so the
    DR k-pair stride is 16B-aligned.
  - PV for q-chunks >= 1: fp8-DR over k-block PAIRS.  exp writes fp8 into
    pair tiles ex[128, 2(jpair), 2(head), 512]; diagonal windows are
    zero-padded (gpsimd memset) so a pair can stream its union window.
Early rows stay accurate on the bf16 path: chunk-0 q/k/v projections are
bf16 (from xTb/w*b), q-chunk 0 attention is the baseline bf16 path with a
bf16 copy of v for k < 512 (v0), protecting rows whose softmax is
concentrated (q < ~128: out ~= v directly, needs better than fp8).

softmax: exp((s)*0.125/64) straight out of PSUM; no max subtraction (scaled
scores are O(+-2), 17 sigma from e4m3's inf at 240 and from Schraudolph's
int8 sign flip).  Causal masking is folded into the scores PSUM on the PE:
diagonal tiles get ONE extra accumulating matmul covering both heads,
scores[k,q] += -BIG * Ustrict[q,k] (lhsT=Ustrict, rhs=-BIG*I), so no
elementwise mask op exists at all.  BIG is 1e9 for true-exp (ACT) tiles
(exp underflows to +0) and 16384 for Schraudolph (DVE) tiles: the int8
conversion saturates at -128 = 0x80 = -0.0 in e4m3 (measured), so masked
weights are exactly -0.0.

Engine split: PE does all matmuls incl. masking; ACT does true exp (fp8/bf16
out) and half the finalize copies; DVE does Schraudolph fast-exp (int8
bit-trick into fp8) for the odd off-diagonal tiles plus q/k/v PSUM->SBUF
copies and the reciprocal; gpsimd (slow, SBUF-only) does pair-pad memsets,
v0->fp8 dup casts and the finalize 1/rowsum muls.

Scheduling: emission is software-pipelined -- scores run one k-tile ahead of
exp, PV-DR lags one PAIR behind its exp (so the PE never stalls on a fresh
exp), score pairs are emitted in runs of two so their 64-row drains overlap
each other instead of the next full-width matmul, the next s-chunk's
projection groups are interleaved at half-chain granularity evenly across
the j loop, and rep r+1's chunk-0 projections fill rep r's last q-chunk.
The whole rep is one flat (qc, hp, j) pipeline -- scores/exp/PV flow across
section boundaries; within each step instructions are emitted in readiness
order (delayed PV, finalize, projection fillers, then the freshest scores
last) because engine queues are strict FIFO and one stalled instruction
blocks everything behind it.  Finalizes are deferred past the next
section's lead-in.

This walrus build only accepts sem waits on EventSemaphore instructions (and
one update on non-DMA instructions), so legalize_sync() post-processes the
Tile-scheduled BIR to hoist waits / split updates, and TileContextPatched
replaces the stock drain-with-eq-wait tail barrier.
"""

import os
import sys

for _p in ("/opt/trn_rl_repo",):
    if _p not in sys.path and os.path.isdir(_p):
        sys.path.append(_p)

import ml_dtypes
import numpy as np

import bass_rust
import concourse.bass as bass
import concourse.mybir as mybir
import concourse.tile as tile
from concourse import library_config
from concourse.bass_utils import run_bass_kernel_spmd
from concourse.masks import make_identity
from concourse.tile import ScopedClock

F32 = mybir.dt.float32
BF16 = mybir.dt.bfloat16
F8 = mybir.dt.float8e4
I8 = mybir.dt.int8
AF = mybir.ActivationFunctionType
DRM = mybir.MatmulPerfMode.DoubleRow
NP_BF16 = ml_dtypes.bfloat16
NP_F8 = ml_dtypes.float8_e4m3


class TileContextPatched(tile.TileContext):
    """Works around this walrus build's 1-sync-wait-per-instruction limit on
    Drain (and the Drain-with-eq-wait barrier form): the tail drain's
    vector-clock waits are re-emitted as individual wait_ge instructions, and
    the engine quiesce/semaphore-reset is done with plain ge-wait semaphores.
    """

    def _drain_and_barrier(self, tick_clock, wait_clock):
        nc = self.nc
        drain = nc.sync.drain()
        wait_clock.add_sem_waits(
            drain.ins, ScopedClock({None: tick_clock.global_clock})
        )
        waits = list(drain.ins.sync_info.on_wait)
        drain.ins.sync_info.on_wait = []
        by_name = {}
        for _k, h in self.sems.allocated().items():
            by_name[getattr(h, "name", str(_k))] = h
        for w in waits:
            h = by_name.get(w.ant_name)
            assert h is not None, f"no handle for sem {w.ant_name}"
            nc.sync.wait_ge(h, w.wait_value)

        done = nc.alloc_semaphore("tile_tail_done")
        go = nc.alloc_semaphore("tile_tail_go")
        n_other = 0
        for _et, eng in nc.engines.items():
            if eng is nc.sync:
                continue
            eng.nop(nofuse=True, hint="tail_done").then_inc(done, 1)
            n_other += 1
        nc.sync.wait_ge(done, n_other)
        nc.sync.nop(nofuse=True, hint="tail_go").then_inc(go, 1)
        nc.gpsimd.wait_ge(go, 1)

        popped = nc._tile_sem_poison_stack.pop()
        assert popped is self._sem_poison
        nc.clear_and_free_semaphores(
            list(self.sems.allocated().values()) + [done, go]
        )


def legalize_sync(nc):
    """Rewrite sync_info to this walrus build's per-instruction limits:
    compute/DMA instructions carry NO waits (hoisted onto preceding
    EventSemaphore instrs, <=2 ge-waits each); non-DMA instructions carry at
    most 1 update (extras move to following EventSemaphores, 1 each, which
    retire only after the preceding same-engine instruction completes).
    DMA updates are never moved (they fire at transfer completion)."""
    for f in nc.m.functions:
        for b in f.blocks:
            changed = False
            new = []
            for inst in b.instructions:
                si = getattr(inst, "sync_info", None)
                if si is None:
                    new.append(inst)
                    continue
                waits = list(si.on_wait)
                upds = list(si.on_update)
                opcode = getattr(inst, "opcode", "") or ""
                is_ev = opcode == "EventSemaphore"
                is_dma = "DMA" in opcode
                max_w = 2 if is_ev else 0
                hoist = []
                if len(waits) > max_w:
                    hoist = waits[max_w:]
                    waits = waits[:max_w]
                extra_upd = []
                max_u = 1 if not is_dma else len(upds)
                if len(upds) > max_u:
                    extra_upd = upds[max_u:]
                    upds = upds[:max_u]
                if hoist or extra_upd:
                    changed = True
                    for i in range(0, len(hoist), 2):
                        ev = mybir.InstEventSemaphore(
                            name=f"evw-{nc.next_id()}", ins=[], outs=[]
                        )
                        ev.engine = inst.engine
                        ev.sync_info = bass_rust.SyncInfo(
                            on_update=[], on_wait=hoist[i : i + 2]
                        )
                        nc.register_instruction(ev)
                        new.append(ev)
                    inst.sync_info = bass_rust.SyncInfo(
                        on_update=upds, on_wait=waits
                    )
                    new.append(inst)
                    for u in extra_upd:
                        ev = mybir.InstEventSemaphore(
                            name=f"evu-{nc.next_id()}", ins=[], outs=[]
                        )
                        ev.engine = inst.engine
                        ev.sync_info = bass_rust.SyncInfo(
                            on_update=[u], on_wait=[]
                        )
                        nc.register_instruction(ev)
                        new.append(ev)
                else:
                    new.append(inst)
            if changed:
                b.instructions = new


# ----------------------------------------------------------------------------
# Problem constants (hardcoded per contest rules)
S = 2048          # sequence length
C = 1024          # embed / qk channels
H_PER_CORE = 4    # heads per core (16 heads / 8 cores * 2 batch-replicas)
D = 64            # head dim
DP = 68           # fp8 v row padded so the DR k-pair stride (4*DP) is %16
DCOLS = H_PER_CORE * D            # 256 weight columns per core
N_CT = C // 128                   # 8 contraction tiles for projections
N_SB = S // 128                   # 16 sequence blocks of 128
QCHUNK = 512
N_QC = S // QCHUNK                # 4 q chunks
N_CORES = 8

W_SCALE = 8.0                     # host-side q/k/v weight scale (fp8 range)
EXP_SCALE = 0.125 / (W_SCALE * W_SCALE)   # folded softmax scale

# Schraudolph fast-exp in fp8e4 bit-space: bits8(exp(y)) ~= 8/ln2 * y +
# (7*8 - 0.35) for y = EXP_SCALE*s.  One DVE tensor_scalar (mult, add) with
# int8 output writes exp directly into the fp8 ex tile.  Max rel err ~7%;
# applied only to off-diagonal tiles of diffuse rows (q >= 512).
SCHRA_A8 = (8.0 / 0.6931471805599453) * EXP_SCALE
SCHRA_B8 = 7.0 * 8.0 - 0.35
BIG_EXACT = 1e9       # mask offset for ACT tiles: exp underflows to +0
# mask offset for DVE Schraudolph tiles: bits = A8*(s-BIG_SAT)+B8 <= -283
# for any score, and the DVE float->int8 conversion SATURATES at -128 =
# 0x80 = -0.0 in e4m3 -> masked weights are exactly -0.0 (measured).
BIG_SAT = 16384.0

DIAG_WIN = {0: (0, 512), 1: (128, 384), 2: (256, 256), 3: (384, 128)}
# DR pair windows for the two diagonal pairs (t in {0,1} and {2,3}):
# union of the member windows
PAIR_WIN = {0: (0, 512), 1: (256, 256)}


def build_program(with_bqk: bool, with_bv: bool, reps: int = 1, ablate=()):
    nc = bass.Bass("TRN2", target_bir_lowering=False, debug=False)

    # bf16 x^T, only s-chunk 0 (feeds the bf16 chunk-0 projections)
    xTb = nc.dram_tensor("xTb", [C, QCHUNK], BF16, kind="ExternalInput").ap()
    # fp8 x^T, full (feeds fp8-DR q/k projections of chunks 1-3 and fp8
    # v projections of st 4-15)
    x8 = nc.dram_tensor("x8", [C, S], F8, kind="ExternalInput").ap()
    wqb = nc.dram_tensor("wqb", [C, DCOLS], BF16, kind="ExternalInput").ap()
    wkb = nc.dram_tensor("wkb", [C, DCOLS], BF16, kind="ExternalInput").ap()
    wvb = nc.dram_tensor("wvb", [C, DCOLS], BF16, kind="ExternalInput").ap()
    wq8 = nc.dram_tensor("wq8", [C, DCOLS], F8, kind="ExternalInput").ap()
    wk8 = nc.dram_tensor("wk8", [C, DCOLS], F8, kind="ExternalInput").ap()
    wv8 = nc.dram_tensor("wv8", [C, DCOLS], F8, kind="ExternalInput").ap()
    bq = nc.dram_tensor("bq", [DCOLS], F32, kind="ExternalInput").ap()
    bk = nc.dram_tensor("bk", [DCOLS], F32, kind="ExternalInput").ap()
    bv = nc.dram_tensor("bv", [DCOLS], F32, kind="ExternalInput").ap()
    # output is produced TRANSPOSED ([DCOLS, S]); the host re-transposes when
    # assembling the full [B, S, C] result (pure layout glue, no math)
    yT = nc.dram_tensor("yT", [DCOLS, S], BF16, kind="ExternalOutput").ap()
    # DRAM bounce for the 1/rowsum row: DMA reads from DRAM may broadcast
    # (stride-0 partition dim), SBUF sources may not
    rrd = nc.dram_tensor("rrd", [8, 2, QCHUNK], BF16, kind="Internal").ap()
    rrd2 = nc.dram_tensor("rrd2", [8, 2, QCHUNK], BF16, kind="Internal").ap()

    with TileContextPatched(nc) as tc:
        with (
            tc.tile_pool(name="singles", bufs=1) as singles,
            tc.tile_pool(name="exp", bufs=10) as exp_pool,
            tc.tile_pool(name="outT", bufs=10) as outT_pool,
            tc.tile_pool(name="rsum", bufs=16) as rsum_pool,
            tc.tile_pool(name="ps_qkv", bufs=2, space="PSUM") as ps_qkv,
            tc.tile_pool(name="ps_sc", bufs=2, space="PSUM") as ps_sc,
            tc.tile_pool(name="ps_po", bufs=2, space="PSUM") as ps_po,
        ):
            # ---- persistent SBUF tensors -----------------------------------
            # x/q/k/v are double-buffered by rep parity so rep r+1's
            # projections overlap rep r's attention tail (the marginal-rep
            # cost is what the harness measures).
            xTb_sb2 = [singles.tile([128, N_CT, QCHUNK], BF16,
                                    name=f"xTb_sb{_i}") for _i in range(2)]
            x8_sb2 = [singles.tile([128, N_CT, S], F8, name=f"x8_sb{_i}")
                      for _i in range(2)]
            wqb_sb = singles.tile([128, N_CT, DCOLS], BF16)
            wkb_sb = singles.tile([128, N_CT, DCOLS], BF16)
            wvb_sb = singles.tile([128, N_CT, DCOLS], BF16)
            wq8_sb = singles.tile([128, N_CT, DCOLS], F8)
            wk8_sb = singles.tile([128, N_CT, DCOLS], F8)
            wv8_sb = singles.tile([128, N_CT, DCOLS], F8)
            qT_sb2 = [singles.tile([128, 2, S], BF16, name=f"qT_sb{_i}")
                      for _i in range(2)]
            kT_sb2 = [singles.tile([128, 2, S], BF16, name=f"kT_sb{_i}")
                      for _i in range(2)]
            # fp8 v (all st blocks), padded to DP cols; ones col at D
            v8_sb2 = [
                singles.tile([128, N_SB, H_PER_CORE, DP], F8,
                             name=f"v8_sb{_i}")
                for _i in range(2)
            ]
            # bf16 v for k < 512 (q-chunk 0 attention)
            v0_sb2 = [
                singles.tile([128, 4, H_PER_CORE, D + 1], BF16,
                             name=f"v0_sb{_i}")
                for _i in range(2)
            ]
            # mask constants: ut[p,f]=1 iff f>p; bi2/bst2 = -BIG*I for
            # both head slices at once (out free dims (2, 128))
            ut = singles.tile([128, 128], BF16, name="ut")
            bi2 = singles.tile([128, 2, 128], BF16, name="bi2")
            bst2 = singles.tile([128, 2, 128], BF16, name="bst2")
            bq_sb = singles.tile([128, 2], F32) if with_bqk else None
            bk_sb = singles.tile([128, 2], F32) if with_bqk else None
            bv_sbT = singles.tile([D, H_PER_CORE], F32, name="bv_sbT") if with_bv else None

            # ---- constants / masks ----------------------------------------
            nc.gpsimd.memset(ut, 1.0)
            nc.gpsimd.affine_select(
                out=ut, in_=ut, compare_op=mybir.AluOpType.is_gt,
                fill=0.0, base=0, pattern=[[1, 128]], channel_multiplier=-1,
            )
            nc.gpsimd.memset(bi2, -BIG_EXACT)
            nc.gpsimd.affine_select(
                out=bi2, in_=bi2, compare_op=mybir.AluOpType.is_equal,
                fill=0.0, base=0, pattern=[[0, 2], [1, 128]],
                channel_multiplier=-1,
            )
            nc.gpsimd.memset(bst2, -BIG_SAT)
            nc.gpsimd.affine_select(
                out=bst2, in_=bst2, compare_op=mybir.AluOpType.is_equal,
                fill=0.0, base=0, pattern=[[0, 2], [1, 128]],
                channel_multiplier=-1,
            )
            for _v in v8_sb2:
                # ones column for PV row sums; zero padding cols D+1..DP
                nc.gpsimd.memset(_v[:, :, :, D : D + 1], 1.0)
                nc.gpsimd.memset(_v[:, :, :, D + 1 : DP], 0.0)
            for _v in v0_sb2:
                nc.vector.memset(_v[:, :, :, D : D + 1], 1.0)

            for _rep in range(reps):
                xTb_sb = xTb_sb2[_rep % 2]
                x8_sb = x8_sb2[_rep % 2]
                qT_sb = qT_sb2[_rep % 2]
                kT_sb = kT_sb2[_rep % 2]
                v8_sb = v8_sb2[_rep % 2]
                v0_sb = v0_sb2[_rep % 2]
                pending_finalize = []
                group_ps = {}

                def emit_x8_dma(sc2):
                    nc.sync.dma_start(
                        out=x8_sb[:, :, 512 * sc2 : 512 * (sc2 + 1)],
                        in_=x8[:, 512 * sc2 : 512 * (sc2 + 1)].rearrange(
                            "(ct p) s -> p ct s", p=128
                        ),
                    )

                def emit_qkv_half(sc2, gi, half, bufs=None):
                    """Half of a projection group.  gi 0..3: qT/kT projection
                    (tensor gi//2, Mtile gi%2); gi 4..7: v block
                    st = 4*sc2 + gi - 4.  qT/kT are [d, s] (Mtile m = heads
                    2m, 2m+1); v is natural [s, d] with the ones column for
                    the PV row sums.  sc2 == 0 is the bf16 path; sc2 >= 1 q/k
                    are fp8-DR, v is fp8 non-DR.  bufs overrides the SBUF
                    buffer set (next-rep chunk-0 groups interleaved into this
                    rep's last q-chunk).  half 0 runs the first half of the
                    contraction chain, half 1 finishes it and emits the
                    PSUM->SBUF copy."""
                    xTb_, x8_, qT_, kT_, v8_, v0_ = (
                        bufs
                        if bufs is not None
                        else (xTb_sb, x8_sb, qT_sb, kT_sb, v8_sb, v0_sb)
                    )
                    if gi < 4:
                        m = gi % 2
                        if half == 0:
                            ps = ps_qkv.tile([128, 512], F32, tag="ps_qkv",
                                             name=f"ps_qk_{sc2}_{gi}")
                            group_ps[(sc2, gi)] = ps
                        else:
                            ps = group_ps.pop((sc2, gi))
                        if sc2 == 0:
                            w_sb, t_sb, b_sb = (
                                (wqb_sb, qT_, bq_sb), (wkb_sb, kT_, bk_sb)
                            )[gi // 2]
                            for ct in range(4 * half, 4 * half + 4):
                                nc.tensor.matmul(
                                    ps,
                                    lhsT=w_sb[:, ct, 128 * m : 128 * (m + 1)],
                                    rhs=xTb_[:, ct, :],
                                    start=(ct == 0),
                                    stop=(ct == N_CT - 1),
                                )
                        else:
                            w_sb, t_sb, b_sb = (
                                (wq8_sb, qT_, bq_sb), (wk8_sb, kT_, bk_sb)
                            )[gi // 2]
                            for cp in range(2 * half, 2 * half + 2):
                                nc.tensor.matmul(
                                    ps,
                                    lhsT=w_sb[:, 2 * cp : 2 * cp + 2,
                                              128 * m : 128 * (m + 1)],
                                    rhs=x8_[:, 2 * cp : 2 * cp + 2,
                                            512 * sc2 : 512 * (sc2 + 1)],
                                    start=(cp == 0),
                                    stop=(cp == N_CT // 2 - 1),
                                    perf_mode=DRM,
                                )
                        if half == 1:
                            dst = t_sb[:, m, 512 * sc2 : 512 * (sc2 + 1)]
                            if with_bqk:
                                if sc2 == 0:
                                    nc.scalar.activation(
                                        dst, ps, AF.Identity,
                                        bias=b_sb[:, m : m + 1],
                                    )
                                else:
                                    nc.vector.tensor_scalar_add(
                                        dst, ps, b_sb[:, m : m + 1],
                                    )
                            else:
                                if sc2 == 0:
                                    nc.scalar.activation(dst, ps, AF.Copy)
                                else:
                                    nc.vector.tensor_copy(dst, ps)
                    else:
                        st = 4 * sc2 + gi - 4
                        if half == 0:
                            ps = ps_qkv.tile([128, DCOLS], F32, tag="ps_qkv",
                                             name=f"ps_v_{sc2}_{gi}")
                            group_ps[(sc2, gi)] = ps
                        else:
                            ps = group_ps.pop((sc2, gi))
                        for ct in range(4 * half, 4 * half + 4):
                            if sc2 == 0:
                                nc.tensor.matmul(
                                    ps,
                                    lhsT=xTb_[:, ct,
                                              128 * st : 128 * (st + 1)],
                                    rhs=wvb_sb[:, ct, :],
                                    start=(ct == 0),
                                    stop=(ct == N_CT - 1),
                                )
                            else:
                                nc.tensor.matmul(
                                    ps,
                                    lhsT=x8_[:, ct,
                                             128 * st : 128 * (st + 1)],
                                    rhs=wv8_sb[:, ct, :],
                                    start=(ct == 0),
                                    stop=(ct == N_CT - 1),
                                )
                        if half == 1:
                            src = ps.rearrange("p (h d) -> p h d",
                                               h=H_PER_CORE)
                            if sc2 == 0:
                                # bf16 v0 from PSUM (undo the 8x weight
                                # scale), then gpsimd dups it into fp8 v8
                                dst0 = v0_[:, st, :, 0:D]
                                nc.scalar.activation(dst0, src, AF.Copy,
                                                     scale=1.0 / W_SCALE)
                                nc.gpsimd.tensor_copy(
                                    v8_[:, st, :, 0:D], dst0
                                )
                            else:
                                dst = v8_[:, st, :, 0:D]
                                if gi % 2 == 0:
                                    nc.scalar.activation(
                                        dst, src, AF.Copy,
                                        scale=1.0 / W_SCALE)
                                else:
                                    nc.vector.tensor_scalar_mul(
                                        dst, src, 1.0 / W_SCALE
                                    )

                def is_schra(qc, j):
                    # odd tiles -> DVE Schraudolph (masked diag entries
                    # saturate to 0x80 = -0.0 via BIG_SAT), even tiles and
                    # all of qc0 -> ACT true exp (-1e9 underflows to +0)
                    return qc > 0 and j % 2 == 1

                def emit_scores(qc, hp, j):
                    t = j - 4 * qc
                    ws, N = (0, 512) if t < 0 else DIAG_WIN[t]
                    q0 = QCHUNK * qc + ws
                    diag = t >= 0 and "mask" not in ablate
                    ps_s = ps_sc.tile([128, 2, 512], F32, tag="ps_sc",
                                      name=f"ps_sc_{qc}_{hp}_{j}")
                    for u in range(2):
                        nc.tensor.matmul(
                            ps_s[:, u, 0:N],
                            lhsT=kT_sb[64 * u : 64 * (u + 1), hp,
                                       128 * j : 128 * (j + 1)],
                            rhs=qT_sb[64 * u : 64 * (u + 1), hp, q0 : q0 + N],
                            start=True,
                            stop=not diag,
                            tile_position=(64 * u, 0),
                        )
                    if diag:
                        # causal mask on the PE: the diagonal 128-block gets
                        # scores[k,q] += -BIG * Ustrict[q,k], both heads in
                        # one 256-col pass
                        nc.tensor.matmul(
                            ps_s[:, :, 0:128],
                            lhsT=ut, rhs=(bst2 if is_schra(qc, j) else bi2),
                            start=False, stop=True,
                            tile_position=(0, 0),
                        )
                    return ps_s, ws, N, t

                def emit_finalize(fqc, fhp, fpo):
                    # Copy each po tile (with its rowsum row D) to SBUF bf16
                    # immediately -- this frees the two po PSUM banks for the
                    # next chunk's PV accumulation.  The 1/rowsum scale then
                    # runs entirely on SBUF tiles, off the critical path: the
                    # rowsum rows bounce through DRAM into a [128, 8] column
                    # layout (parallel DVE reciprocal), bounce back, and are
                    # broadcast-read (stride-0 partition dim; DRAM sources
                    # only).
                    slot = 2 * fqc + fhp
                    ots = []
                    for u in range(2):
                        ot = outT_pool.tile([D + 1, QCHUNK], BF16, tag="ot",
                                            name=f"ot_{fqc}_{fhp}_{u}")
                        if u == 0:
                            nc.scalar.activation(ot, fpo[u], AF.Copy)
                        else:
                            nc.vector.tensor_copy(ot, fpo[u])
                        ots.append(ot)
                        nc.sync.dma_start(
                            out=rrd[slot, u, :], in_=ot[D : D + 1, :]
                        )
                    rc = rsum_pool.tile([128, 8], BF16, tag="rc",
                                        name=f"rc_{fqc}_{fhp}")
                    rc2 = rsum_pool.tile([128, 8], BF16, tag="rc2",
                                         name=f"rc2_{fqc}_{fhp}")
                    nc.sync.dma_start(
                        out=rc,
                        in_=bass.AP(rrd.tensor, slot * 2 * QCHUNK,
                                    [[8, 128], [1, 8]]),
                    )
                    with nc.allow_low_precision(reason="bf16 softmax scale"):
                        nc.vector.reciprocal(rc2, rc)
                    nc.sync.dma_start(
                        out=bass.AP(rrd2.tensor, slot * 2 * QCHUNK,
                                    [[8, 128], [1, 8]]),
                        in_=rc2,
                    )
                    for u in range(2):
                        h = 2 * fhp + u
                        bc = outT_pool.tile([D, QCHUNK], BF16, tag="bc",
                                            name=f"bc_{fqc}_{fhp}_{u}")
                        nc.sync.dma_start(
                            out=bc,
                            in_=bass.AP(rrd2.tensor,
                                        (slot * 2 + u) * QCHUNK,
                                        [[0, D], [1, QCHUNK]]),
                        )
                        yt = outT_pool.tile([D, QCHUNK], BF16, tag="yt",
                                            name=f"yt_{fqc}_{fhp}_{u}")
                        nc.gpsimd.tensor_mul(yt, ots[u][0:D, :], bc)
                        if with_bv:
                            nc.gpsimd.tensor_scalar_add(
                                yt, yt, bv_sbT[:, h : h + 1]
                            )
                        nc.sync.dma_start(
                            out=yT[D * h : D * (h + 1),
                                   QCHUNK * fqc : QCHUNK * (fqc + 1)],
                            in_=yt,
                        )

                # weights are rep-invariant: loaded once before rep 0.
                # x chunks 0/1 for rep r>0 were prefetched during rep r-1.
                if _rep == 0:
                    wqb_r = wqb.rearrange("(ct p) o -> p ct o", p=128)
                    xb_r = xTb.rearrange("(ct p) s -> p ct s", p=128)
                    nc.sync.dma_start(out=wqb_sb[:, 0:4, :], in_=wqb_r[:, 0:4, :])
                    nc.sync.dma_start(out=xTb_sb[:, 0:4, :], in_=xb_r[:, 0:4, :])
                    nc.sync.dma_start(out=wqb_sb[:, 4:8, :], in_=wqb_r[:, 4:8, :])
                    nc.sync.dma_start(out=xTb_sb[:, 4:8, :], in_=xb_r[:, 4:8, :])
                    nc.sync.dma_start(
                        out=wkb_sb, in_=wkb.rearrange("(ct p) o -> p ct o", p=128)
                    )
                    nc.sync.dma_start(
                        out=wvb_sb, in_=wvb.rearrange("(ct p) o -> p ct o", p=128)
                    )
                    nc.sync.dma_start(
                        out=wq8_sb, in_=wq8.rearrange("(ct p) o -> p ct o", p=128)
                    )
                    nc.sync.dma_start(
                        out=wk8_sb, in_=wk8.rearrange("(ct p) o -> p ct o", p=128)
                    )
                    nc.sync.dma_start(
                        out=wv8_sb, in_=wv8.rearrange("(ct p) o -> p ct o", p=128)
                    )
                    if with_bqk:
                        nc.sync.dma_start(out=bq_sb, in_=bq.rearrange("(m p) -> p m", p=128))
                        nc.sync.dma_start(out=bk_sb, in_=bk.rearrange("(m p) -> p m", p=128))
                    if with_bv:
                        nc.sync.dma_start(
                            out=bv_sbT,
                            in_=bv.rearrange("(h d) -> d h", h=H_PER_CORE),
                        )
                    if N_QC > 1:
                        emit_x8_dma(1)
                    # rep 0's s-chunk 0 projections run up front; for later
                    # reps they were interleaved into rep r-1's last q-chunk
                    for gi in range(8):
                        emit_qkv_half(0, gi, 0)
                        emit_qkv_half(0, gi, 1)

                xTb_next = xTb_sb2[(_rep + 1) % 2]
                x8_next = x8_sb2[(_rep + 1) % 2]
                next_bufs = (
                    xTb_next, x8_next,
                    qT_sb2[(_rep + 1) % 2], kT_sb2[(_rep + 1) % 2],
                    v8_sb2[(_rep + 1) % 2], v0_sb2[(_rep + 1) % 2],
                )
                def qc_preamble(qc):
                    if qc + 2 < N_QC:
                        emit_x8_dma(qc + 2)
                    elif _rep + 1 < reps:
                        # prefetch next rep's bf16 chunk 0 / fp8 chunk 1 into
                        # the other buffer (fp8 chunk 0 is never read)
                        sc2 = qc - 2
                        if sc2 == 0:
                            nc.sync.dma_start(
                                out=xTb_next,
                                in_=xTb.rearrange("(ct p) s -> p ct s", p=128),
                            )
                        else:
                            nc.sync.dma_start(
                                out=x8_next[:, :, 512 * sc2 : 512 * (sc2 + 1)],
                                in_=x8[:, 512 * sc2 : 512 * (sc2 + 1)].rearrange(
                                    "(ct p) s -> p ct s", p=128
                                ),
                            )
                    # alternate q/k and v groups so the PSUM->SBUF copy
                    # load (DVE/ACT) and the filler matmul shapes spread
                    # evenly across the chunk
                    GI_ORDER = (0, 4, 1, 5, 2, 6, 3, 7)
                    if qc + 1 < N_QC:
                        return [(qc + 1, gi, h, None)
                                for gi in GI_ORDER for h in range(2)]
                    if _rep + 1 < reps:
                        # next rep's chunk-0 projections fill qc=3's bubbles
                        return [(0, gi, h, next_bufs)
                                for gi in GI_ORDER for h in range(2)]
                    return []

                po_by = {}
                extiles = {}
                pvq = []
                interleave = []
                ilen0 = islot = n_slots = 0

                def emit_pv(entry, last):
                    fqc, fhp, fkey, fex, fws, fN = entry
                    fpo = po_by[(fqc, fhp)]
                    if fqc == 0:
                        for u in range(2):
                            nc.tensor.matmul(
                                fpo[u][:, fws : fws + fN],
                                lhsT=v0_sb[:, fkey, 2 * fhp + u, :],
                                rhs=fex[:, u, 0:fN],
                                start=(fkey == 0),
                                stop=last,
                            )
                    else:
                        for u in range(2):
                            nc.tensor.matmul(
                                fpo[u][:, fws : fws + fN],
                                lhsT=v8_sb[:, 2 * fkey : 2 * fkey + 2,
                                           2 * fhp + u, 0 : D + 1],
                                rhs=fex[:, :, u, fws : fws + fN],
                                start=(fkey == 0),
                                stop=last,
                                perf_mode=DRM,
                            )

                all_steps = [
                    (qc, hp, j)
                    for qc in range(N_QC if "attn" not in ablate else 0)
                    for hp in range(2)
                    for j in range(4 * qc + 4)
                ]
                pipeline = [emit_scores(0, 0, 0)] if all_steps else []
                next_s = 1
                for si, (qc, hp, j) in enumerate(all_steps):
                    jmax = 4 * qc + 4
                    if hp == 0 and j == 0:
                        interleave = qc_preamble(qc)
                        ilen0 = len(interleave)
                        islot = 0
                        n_slots = jmax * 2
                    if j == 0:
                        po_by[(qc, hp)] = [
                            ps_po.tile([D + 1, QCHUNK], F32, tag="ps_po",
                                       name=f"po_{qc}_{hp}_{u}")
                            for u in range(2)
                        ]
                    # the previous section's delayed last pair must land
                    # before its finalize is emitted below
                    while pvq and (pvq[0][0], pvq[0][1]) != (qc, hp):
                        emit_pv(pvq.pop(0), last=True)
                    ps_s, ws, N, t = pipeline.pop(0)
                    if qc == 0:
                        ex = exp_pool.tile([128, 2, 512], BF16, tag="ex",
                                           name=f"ex_{qc}_{hp}_{j}")
                        nc.scalar.activation(
                            ex[:, :, 0:N], ps_s[:, :, 0:N], AF.Exp,
                            scale=EXP_SCALE,
                        )
                    else:
                        pr = j // 2
                        if j % 2 == 0:
                            extiles[(hp, pr)] = exp_pool.tile(
                                [128, 2, 2, 512], F8, tag="ex",
                                name=f"ex_{qc}_{hp}_{pr}",
                            )
                        ex = extiles[(hp, pr)]
                        slab = ex[:, j % 2, :, ws : ws + N]
                        # engine split: DVE Schraudolph on odd tiles (masked
                        # entries saturate to -0.0), ACT true exp elsewhere
                        if not is_schra(qc, j):
                            nc.scalar.activation(
                                slab, ps_s[:, :, 0:N], AF.Exp,
                                scale=EXP_SCALE,
                            )
                        else:
                            nc.vector.tensor_scalar(
                                slab.bitcast(I8),
                                ps_s[:, :, 0:N],
                                SCHRA_A8, SCHRA_B8,
                                mybir.AluOpType.mult,
                                mybir.AluOpType.add,
                            )
                    # queue this step's PV pair, then emit in readiness
                    # order: delayed PV (oldest exp dep) first, finalize +
                    # fillers, and the NEXT scores last -- it waits on the
                    # freshest exp (ps_sc buffer rotation), so anything
                    # behind it in the PE FIFO would stall with it
                    if qc == 0:
                        pvq.append((qc, hp, j, ex, ws, N))
                    else:
                        if t >= 0:
                            # zero the pair-window pad left of this tile's
                            # own window
                            pws, pN = PAIR_WIN[t // 2]
                            if ws > pws:
                                nc.gpsimd.memset(
                                    ex[:, j % 2, :, pws:ws], 0.0
                                )
                        if j % 2 == 1:
                            if t >= 0:
                                pws, pN = PAIR_WIN[t // 2]
                            else:
                                pws, pN = 0, 512
                            pvq.append((qc, hp, pr, ex, pws, pN))
                            del extiles[(hp, pr)]
                    if len(pvq) > 1 and (pvq[0][0], pvq[0][1]) == (qc, hp):
                        emit_pv(pvq.pop(0), last=False)
                    if j == 1 and pending_finalize:
                        emit_finalize(*pending_finalize.pop(0))
                    islot += 1
                    while interleave and len(interleave) > (
                        ilen0 * (n_slots - islot) // n_slots
                    ):
                        emit_qkv_half(*interleave.pop(0))
                    # emit the next scores; at pair completions emit TWO
                    # adjacent score pairs so their 64-row drains overlap
                    # each other instead of the next full-width MM
                    extra = 1 if (qc > 0 and j % 2 == 1) else 0
                    while next_s < len(all_steps) and next_s <= si + 1 + extra:
                        pipeline.append(emit_scores(*all_steps[next_s]))
                        next_s += 1
                    if j == jmax - 1 and "finalize" not in ablate:
                        pending_finalize.append((qc, hp, po_by[(qc, hp)]))
                while pvq:
                    emit_pv(pvq.pop(0), last=True)
                while interleave:
                    emit_qkv_half(*interleave.pop(0))
                while pending_finalize:
                    emit_finalize(*pending_finalize.pop(0))
    legalize_sync(nc)
    return nc


_CACHE = {}


def get_program(with_bqk: bool, with_bv: bool, reps: int = 1):
    key = (with_bqk, with_bv, reps)
    if key not in _CACHE:
        _CACHE[key] = build_program(with_bqk, with_bv, reps)
    return _CACHE[key]


def make_in_maps(x, Wqk, bqk, Wv, bv):
    x = np.asarray(x, dtype=np.float32)
    Wqk = np.asarray(Wqk, dtype=np.float32)
    bqk = np.asarray(bqk, dtype=np.float32)
    Wv = np.asarray(Wv, dtype=np.float32)
    bv = np.asarray(bv, dtype=np.float32)
    xT = [np.ascontiguousarray(x[b].T) for b in range(x.shape[0])]
    in_maps = []
    for c in range(N_CORES):
        b, g = divmod(c, 4)
        cols = slice(DCOLS * g, DCOLS * (g + 1))
        wq = np.ascontiguousarray(Wqk[:, :C][:, cols]) * W_SCALE
        wk = np.ascontiguousarray(Wqk[:, C:][:, cols]) * W_SCALE
        wv = np.ascontiguousarray(Wv[:, cols]) * W_SCALE
        in_maps.append(
            {
                "xTb": xT[b][:, 0:QCHUNK].astype(NP_BF16),
                "x8": xT[b].astype(NP_F8),
                "wqb": wq.astype(NP_BF16),
                "wkb": wk.astype(NP_BF16),
                "wvb": wv.astype(NP_BF16),
                "wq8": wq.astype(NP_F8),
                "wk8": wk.astype(NP_F8),
                "wv8": wv.astype(NP_F8),
                "bq": np.ascontiguousarray(bqk[:C][cols]) * W_SCALE,
                "bk": np.ascontiguousarray(bqk[C:][cols]) * W_SCALE,
                "bv": np.ascontiguousarray(bv[cols]),
            }
        )
    return in_maps


def assemble_output(results, B):
    y = np.empty((B, S, C), dtype=np.float32)
    for c in range(N_CORES):
        b, g = divmod(c, 4)
        y[b, :, DCOLS * g : DCOLS * (g + 1)] = (
            np.asarray(results[c]["yT"]).astype(np.float32).T
        )
    return y


def kernel(x, Wqk, bqk, Wv, bv):
    in_maps = make_in_maps(x, Wqk, bqk, Wv, bv)
    with_bqk = bool(np.any(np.asarray(bqk) != 0))
    with_bv = bool(np.any(np.asarray(bv) != 0))
    nc = get_program(with_bqk, with_bv)
    res = run_bass_kernel_spmd(nc, in_maps, core_ids=list(range(N_CORES)))
    return assemble_output(res.results, np.asarray(x).shape[0])


if __name__ == "__main__":
    rng = np.random.default_rng(0)
    x = rng.standard_normal((2, S, C), dtype=np.float32)
    Wqk = rng.standard_normal((C, 2 * C), dtype=np.float32) * 0.02
    bqk = np.zeros((2 * C,), dtype=np.float32)
    Wv = rng.standard_normal((C, C), dtype=np.float32) * 0.02
    bv = np.zeros((C,), dtype=np.float32)
    out = kernel(x, Wqk, bqk, Wv, bv)
    print("kernel output:", out.shape, out.dtype, float(np.abs(out).max()))


# revision 41
# speedup vs baseline: 1.0922x; 1.0922x over previous
"""Multi-head causal self-attention (B=2, S=2048, C=1024, H=16) on 8 TRN2
NeuronCores.

Sharding: data-parallel over batch x tensor-parallel over heads.  Core c
handles batch b = c//4 and the 4 heads g = c%4 -> heads [4g, 4g+4).  Each core
computes its QKV projections from the full x[b] (weights column-sharded
head-wise), runs causal attention for its 4 heads, and writes a [S, 256]
output shard.  No cross-device communication.

Numerics: mixed bf16/fp8.  fp8e4 (TRN E4M3, max 240) matmuls with
perf_mode=DoubleRow stream 2 contraction-tiles per pass (measured 216ns/MM at
N=512 - exactly 2x bf16 throughput, LDWEIGHTS fully hidden).  fp8 error
averages out over diffuse softmax rows (q >= 512 has >= 513-way softmax, max
weight ~1%), so everything fp8-able for those rows is fp8:
  - q/k projections for s-chunks 1-3: fp8-DR (inputs x8 = fp8(x^T),
    wq8/wk8 = fp8(8*W); the 8x weight scale keeps W ~N(0,0.02) out of e4m3's
    subnormal range; exp scale absorbs the 64x on scores).
  - v projection st>=4 (k >= 512): fp8 operands (bf16-rate non-DR matmuls),
    v stored fp8 with a ones column (PV row sums) padded to 68 cols so the
    DR k-pair stride is 16B-aligned.
  - PV for q-chunks >= 1: fp8-DR over k-block PAIRS.  exp writes fp8 into
    pair tiles ex[128, 2(jpair), 2(head), 512]; diagonal windows are
    zero-padded (gpsimd memset) so a pair can stream its union window.
Early rows stay accurate on the bf16 path: chunk-0 q/k/v projections are
bf16 (from xTb/w*b), q-chunk 0 attention is the baseline bf16 path with a
bf16 copy of v for k < 512 (v0), protecting rows whose softmax is
concentrated (q < ~128: out ~= v directly, needs better than fp8).

softmax: exp((s)*0.125/64) straight out of PSUM; no max subtraction (scaled
scores are O(+-2), 17 sigma from e4m3's inf at 240 and from Schraudolph's
int8 sign flip).  Causal masking is folded into the scores PSUM on the PE:
diagonal tiles get ONE extra accumulating matmul covering both heads,
scores[k,q] += -BIG * Ustrict[q,k] (lhsT=Ustrict, rhs=-BIG*I), so no
elementwise mask op exists at all.  BIG is 1e9 for true-exp (ACT) tiles
(exp underflows to +0) and 16384 for Schraudolph (DVE) tiles: the int8
conversion saturates at -128 = 0x80 = -0.0 in e4m3 (measured), so masked
weights are exactly -0.0.

Engine split: PE does all matmuls incl. masking; ACT does true exp (fp8/bf16
out) and half the finalize copies; DVE does Schraudolph fast-exp (int8
bit-trick into fp8) for the odd off-diagonal tiles plus q/k/v PSUM->SBUF
copies and the reciprocal; gpsimd (slow, SBUF-only) does pair-pad memsets,
v0->fp8 dup casts and the finalize 1/rowsum muls.

Scheduling: emission is software-pipelined -- scores run one k-tile ahead of
exp, PV-DR lags one PAIR behind its exp (so the PE never stalls on a fresh
exp), score pairs are emitted in runs of two so their 64-row drains overlap
each other instead of the next full-width matmul, the next s-chunk's
projection groups are interleaved at half-chain granularity evenly across
the j loop, and rep r+1's chunk-0 projections fill rep r's last q-chunk.
The whole rep is one flat (qc, hp, j) pipeline -- scores/exp/PV flow across
section boundaries; within each step instructions are emitted in readiness
order (delayed PV, finalize, projection fillers, then the freshest scores
last) because engine queues are strict FIFO and one stalled instruction
blocks everything behind it.  Finalizes are deferred past the next
section's lead-in.

This walrus build only accepts sem waits on EventSemaphore instructions (and
one update on non-DMA instructions), so legalize_sync() post-processes the
Tile-scheduled BIR to hoist waits / split updates, and TileContextPatched
replaces the stock drain-with-eq-wait tail barrier.
"""

import os
import sys

for _p in ("/opt/trn_rl_repo",):
    if _p not in sys.path and os.path.isdir(_p):
        sys.path.append(_p)

import ml_dtypes
import numpy as np

import bass_rust
import concourse.bass as bass
import concourse.mybir as mybir
import concourse.tile as tile
from concourse import library_config
from concourse.bass_utils import run_bass_kernel_spmd
from concourse.masks import make_identity
from concourse.tile import ScopedClock

F32 = mybir.dt.float32
BF16 = mybir.dt.bfloat16
F8 = mybir.dt.float8e4
I8 = mybir.dt.int8
AF = mybir.ActivationFunctionType
DRM = mybir.MatmulPerfMode.DoubleRow
NP_BF16 = ml_dtypes.bfloat16
NP_F8 = ml_dtypes.float8_e4m3


class TileContextPatched(tile.TileContext):
    """Works around this walrus build's 1-sync-wait-per-instruction limit on
    Drain (and the Drain-with-eq-wait barrier form): the tail drain's
    vector-clock waits are re-emitted as individual wait_ge instructions, and
    the engine quiesce/semaphore-reset is done with plain ge-wait semaphores.
    """

    def _drain_and_barrier(self, tick_clock, wait_clock):
        nc = self.nc
        drain = nc.sync.drain()
        wait_clock.add_sem_waits(
            drain.ins, ScopedClock({None: tick_clock.global_clock})
        )
        waits = list(drain.ins.sync_info.on_wait)
        drain.ins.sync_info.on_wait = []
        by_name = {}
        for _k, h in self.sems.allocated().items():
            by_name[getattr(h, "name", str(_k))] = h
        for w in waits:
            h = by_name.get(w.ant_name)
            assert h is not None, f"no handle for sem {w.ant_name}"
            nc.sync.wait_ge(h, w.wait_value)

        done = nc.alloc_semaphore("tile_tail_done")
        go = nc.alloc_semaphore("tile_tail_go")
        n_other = 0
        for _et, eng in nc.engines.items():
            if eng is nc.sync:
                continue
            eng.nop(nofuse=True, hint="tail_done").then_inc(done, 1)
            n_other += 1
        nc.sync.wait_ge(done, n_other)
        nc.sync.nop(nofuse=True, hint="tail_go").then_inc(go, 1)
        nc.gpsimd.wait_ge(go, 1)

        popped = nc._tile_sem_poison_stack.pop()
        assert popped is self._sem_poison
        nc.clear_and_free_semaphores(
            list(self.sems.allocated().values()) + [done, go]
        )


def legalize_sync(nc):
    """Rewrite sync_info to this walrus build's per-instruction limits:
    compute/DMA instructions carry NO waits (hoisted onto preceding
    EventSemaphore instrs, <=2 ge-waits each); non-DMA instructions carry at
    most 1 update (extras move to following EventSemaphores, 1 each, which
    retire only after the preceding same-engine instruction completes).
    DMA updates are never moved (they fire at transfer completion)."""
    for f in nc.m.functions:
        for b in f.blocks:
            changed = False
            new = []
            for inst in b.instructions:
                si = getattr(inst, "sync_info", None)
                if si is None:
                    new.append(inst)
                    continue
                waits = list(si.on_wait)
                upds = list(si.on_update)
                opcode = getattr(inst, "opcode", "") or ""
                is_ev = opcode == "EventSemaphore"
                is_dma = "DMA" in opcode
                max_w = 2 if is_ev else 0
                hoist = []
                if len(waits) > max_w:
                    hoist = waits[max_w:]
                    waits = waits[:max_w]
                extra_upd = []
                max_u = 1 if not is_dma else len(upds)
                if len(upds) > max_u:
                    extra_upd = upds[max_u:]
                    upds = upds[:max_u]
                if hoist or extra_upd:
                    changed = True
                    for i in range(0, len(hoist), 2):
                        ev = mybir.InstEventSemaphore(
                            name=f"evw-{nc.next_id()}", ins=[], outs=[]
                        )
                        ev.engine = inst.engine
                        ev.sync_info = bass_rust.SyncInfo(
                            on_update=[], on_wait=hoist[i : i + 2]
                        )
                        nc.register_instruction(ev)
                        new.append(ev)
                    inst.sync_info = bass_rust.SyncInfo(
                        on_update=upds, on_wait=waits
                    )
                    new.append(inst)
                    for u in extra_upd:
                        ev = mybir.InstEventSemaphore(
                            name=f"evu-{nc.next_id()}", ins=[], outs=[]
                        )
                        ev.engine = inst.engine
                        ev.sync_info = bass_rust.SyncInfo(
                            on_update=[u], on_wait=[]
                        )
                        nc.register_instruction(ev)
                        new.append(ev)
                else:
                    new.append(inst)
            if changed:
                b.instructions = new


# ----------------------------------------------------------------------------
# Problem constants (hardcoded per contest rules)
S = 2048          # sequence length
C = 1024          # embed / qk channels
H_PER_CORE = 4    # heads per core (16 heads / 8 cores * 2 batch-replicas)
D = 64            # head dim
DP = 68           # fp8 v row padded so the DR k-pair stride (4*DP) is %16
DCOLS = H_PER_CORE * D            # 256 weight columns per core
N_CT = C // 128                   # 8 contraction tiles for projections
N_SB = S // 128                   # 16 sequence blocks of 128
QCHUNK = 512
N_QC = S // QCHUNK                # 4 q chunks
N_CORES = 8

W_SCALE = 8.0                     # host-side q/k/v weight scale (fp8 range)
EXP_SCALE = 0.125 / (W_SCALE * W_SCALE)   # folded softmax scale

# Schraudolph fast-exp in fp8e4 bit-space: bits8(exp(y)) ~= 8/ln2 * y +
# (7*8 - 0.35) for y = EXP_SCALE*s.  One DVE tensor_scalar (mult, add) with
# int8 output writes exp directly into the fp8 ex tile.  Max rel err ~7%;
# applied only to off-diagonal tiles of diffuse rows (q >= 512).
SCHRA_A8 = (8.0 / 0.6931471805599453) * EXP_SCALE
SCHRA_B8 = 7.0 * 8.0 - 0.35
BIG_EXACT = 1e9       # mask offset for ACT tiles: exp underflows to +0
# mask offset for DVE Schraudolph tiles: bits = A8*(s-BIG_SAT)+B8 <= -283
# for any score, and the DVE float->int8 conversion SATURATES at -128 =
# 0x80 = -0.0 in e4m3 -> masked weights are exactly -0.0 (measured).
BIG_SAT = 16384.0

DIAG_WIN = {0: (0, 512), 1: (128, 384), 2: (256, 256), 3: (384, 128)}
# DR pair windows for the two diagonal pairs (t in {0,1} and {2,3}):
# union of the member windows
PAIR_WIN = {0: (0, 512), 1: (256, 256)}


def build_program(with_bqk: bool, with_bv: bool, reps: int = 1, ablate=()):
    nc = bass.Bass("TRN2", target_bir_lowering=False, debug=False)

    # bf16 x^T, only s-chunk 0 (feeds the bf16 chunk-0 projections)
    xTb = nc.dram_tensor("xTb", [C, QCHUNK], BF16, kind="ExternalInput").ap()
    # fp8 x^T, full (feeds fp8-DR q/k projections of chunks 1-3 and fp8
    # v projections of st 4-15)
    x8 = nc.dram_tensor("x8", [C, S], F8, kind="ExternalInput").ap()
    wqb = nc.dram_tensor("wqb", [C, DCOLS], BF16, kind="ExternalInput").ap()
    wkb = nc.dram_tensor("wkb", [C, DCOLS], BF16, kind="ExternalInput").ap()
    wvb = nc.dram_tensor("wvb", [C, DCOLS], BF16, kind="ExternalInput").ap()
    wq8 = nc.dram_tensor("wq8", [C, DCOLS], F8, kind="ExternalInput").ap()
    wk8 = nc.dram_tensor("wk8", [C, DCOLS], F8, kind="ExternalInput").ap()
    wv8 = nc.dram_tensor("wv8", [C, DCOLS], F8, kind="ExternalInput").ap()
    bq = nc.dram_tensor("bq", [DCOLS], F32, kind="ExternalInput").ap()
    bk = nc.dram_tensor("bk", [DCOLS], F32, kind="ExternalInput").ap()
    bv = nc.dram_tensor("bv", [DCOLS], F32, kind="ExternalInput").ap()
    # output is produced TRANSPOSED ([DCOLS, S]); the host re-transposes when
    # assembling the full [B, S, C] result (pure layout glue, no math)
    yT = nc.dram_tensor("yT", [DCOLS, S], BF16, kind="ExternalOutput").ap()
    # DRAM bounce for the 1/rowsum row: DMA reads from DRAM may broadcast
    # (stride-0 partition dim), SBUF sources may not
    rrd = nc.dram_tensor("rrd", [8, 2, QCHUNK], BF16, kind="Internal").ap()
    rrd2 = nc.dram_tensor("rrd2", [8, 2, QCHUNK], BF16, kind="Internal").ap()

    with TileContextPatched(nc) as tc:
        with (
            tc.tile_pool(name="singles", bufs=1) as singles,
            tc.tile_pool(name="exp", bufs=10) as exp_pool,
            tc.tile_pool(name="outT", bufs=10) as outT_pool,
            tc.tile_pool(name="rsum", bufs=16) as rsum_pool,
            tc.tile_pool(name="ps_qkv", bufs=2, space="PSUM") as ps_qkv,
            tc.tile_pool(name="ps_sc", bufs=2, space="PSUM") as ps_sc,
            tc.tile_pool(name="ps_po", bufs=2, space="PSUM") as ps_po,
        ):
            # ---- persistent SBUF tensors -----------------------------------
            # x/q/k/v are double-buffered by rep parity so rep r+1's
            # projections overlap rep r's attention tail (the marginal-rep
            # cost is what the harness measures).
            xTb_sb2 = [singles.tile([128, N_CT, QCHUNK], BF16,
                                    name=f"xTb_sb{_i}") for _i in range(2)]
            x8_sb2 = [singles.tile([128, N_CT, S], F8, name=f"x8_sb{_i}")
                      for _i in range(2)]
            wqb_sb = singles.tile([128, N_CT, DCOLS], BF16)
            wkb_sb = singles.tile([128, N_CT, DCOLS], BF16)
            wvb_sb = singles.tile([128, N_CT, DCOLS], BF16)
            wq8_sb = singles.tile([128, N_CT, DCOLS], F8)
            wk8_sb = singles.tile([128, N_CT, DCOLS], F8)
            wv8_sb = singles.tile([128, N_CT, DCOLS], F8)
            qT_sb2 = [singles.tile([128, 2, S], BF16, name=f"qT_sb{_i}")
                      for _i in range(2)]
            kT_sb2 = [singles.tile([128, 2, S], BF16, name=f"kT_sb{_i}")
                      for _i in range(2)]
            # fp8 v (all st blocks), padded to DP cols; ones col at D
            v8_sb2 = [
                singles.tile([128, N_SB, H_PER_CORE, DP], F8,
                             name=f"v8_sb{_i}")
                for _i in range(2)
            ]
            # bf16 v for k < 512 (q-chunk 0 attention)
            v0_sb2 = [
                singles.tile([128, 4, H_PER_CORE, D + 1], BF16,
                             name=f"v0_sb{_i}")
                for _i in range(2)
            ]
            # mask constants: ut[p,f]=1 iff f>p; bi2/bst2 = -BIG*I for
            # both head slices at once (out free dims (2, 128))
            ut = singles.tile([128, 128], BF16, name="ut")
            bi2 = singles.tile([128, 2, 128], BF16, name="bi2")
            bst2 = singles.tile([128, 2, 128], BF16, name="bst2")
            bq_sb = singles.tile([128, 2], F32) if with_bqk else None
            bk_sb = singles.tile([128, 2], F32) if with_bqk else None
            bv_sbT = singles.tile([D, H_PER_CORE], F32, name="bv_sbT") if with_bv else None

            # ---- constants / masks ----------------------------------------
            nc.gpsimd.memset(ut, 1.0)
            nc.gpsimd.affine_select(
                out=ut, in_=ut, compare_op=mybir.AluOpType.is_gt,
                fill=0.0, base=0, pattern=[[1, 128]], channel_multiplier=-1,
            )
            nc.gpsimd.memset(bi2, -BIG_EXACT)
            nc.gpsimd.affine_select(
                out=bi2, in_=bi2, compare_op=mybir.AluOpType.is_equal,
                fill=0.0, base=0, pattern=[[0, 2], [1, 128]],
                channel_multiplier=-1,
            )
            nc.gpsimd.memset(bst2, -BIG_SAT)
            nc.gpsimd.affine_select(
                out=bst2, in_=bst2, compare_op=mybir.AluOpType.is_equal,
                fill=0.0, base=0, pattern=[[0, 2], [1, 128]],
                channel_multiplier=-1,
            )
            for _v in v8_sb2:
                # ones column for PV row sums; zero padding cols D+1..DP
                nc.gpsimd.memset(_v[:, :, :, D : D + 1], 1.0)
                nc.gpsimd.memset(_v[:, :, :, D + 1 : DP], 0.0)
            for _v in v0_sb2:
                nc.vector.memset(_v[:, :, :, D : D + 1], 1.0)

            for _rep in range(reps):
                xTb_sb = xTb_sb2[_rep % 2]
                x8_sb = x8_sb2[_rep % 2]
                qT_sb = qT_sb2[_rep % 2]
                kT_sb = kT_sb2[_rep % 2]
                v8_sb = v8_sb2[_rep % 2]
                v0_sb = v0_sb2[_rep % 2]
                pending_finalize = []
                group_ps = {}

                def emit_x8_dma(sc2):
                    nc.sync.dma_start(
                        out=x8_sb[:, :, 512 * sc2 : 512 * (sc2 + 1)],
                        in_=x8[:, 512 * sc2 : 512 * (sc2 + 1)].rearrange(
                            "(ct p) s -> p ct s", p=128
                        ),
                    )

                def emit_qkv_half(sc2, gi, half, bufs=None):
                    """Half of a projection group.  gi 0..3: qT/kT projection
                    (tensor gi//2, Mtile gi%2); gi 4..7: v block
                    st = 4*sc2 + gi - 4.  qT/kT are [d, s] (Mtile m = heads
                    2m, 2m+1); v is natural [s, d] with the ones column for
                    the PV row sums.  sc2 == 0 is the bf16 path; sc2 >= 1 q/k
                    are fp8-DR, v is fp8 non-DR.  bufs overrides the SBUF
                    buffer set (next-rep chunk-0 groups interleaved into this
                    rep's last q-chunk).  half 0 runs the first half of the
                    contraction chain, half 1 finishes it and emits the
                    PSUM->SBUF copy."""
                    xTb_, x8_, qT_, kT_, v8_, v0_ = (
                        bufs
                        if bufs is not None
                        else (xTb_sb, x8_sb, qT_sb, kT_sb, v8_sb, v0_sb)
                    )
                    if gi < 4:
                        m = gi % 2
                        if half == 0:
                            ps = ps_qkv.tile([128, 512], F32, tag="ps_qkv",
                                             name=f"ps_qk_{sc2}_{gi}")
                            group_ps[(sc2, gi)] = ps
                        else:
                            ps = group_ps.pop((sc2, gi))
                        if sc2 == 0:
                            w_sb, t_sb, b_sb = (
                                (wqb_sb, qT_, bq_sb), (wkb_sb, kT_, bk_sb)
                            )[gi // 2]
                            for ct in range(4 * half, 4 * half + 4):
                                nc.tensor.matmul(
                                    ps,
                                    lhsT=w_sb[:, ct, 128 * m : 128 * (m + 1)],
                                    rhs=xTb_[:, ct, :],
                                    start=(ct == 0),
                                    stop=(ct == N_CT - 1),
                                )
                        else:
                            w_sb, t_sb, b_sb = (
                                (wq8_sb, qT_, bq_sb), (wk8_sb, kT_, bk_sb)
                            )[gi // 2]
                            for cp in range(2 * half, 2 * half + 2):
                                nc.tensor.matmul(
                                    ps,
                                    lhsT=w_sb[:, 2 * cp : 2 * cp + 2,
                                              128 * m : 128 * (m + 1)],
                                    rhs=x8_[:, 2 * cp : 2 * cp + 2,
                                            512 * sc2 : 512 * (sc2 + 1)],
                                    start=(cp == 0),
                                    stop=(cp == N_CT // 2 - 1),
                                    perf_mode=DRM,
                                )
                        if half == 1:
                            dst = t_sb[:, m, 512 * sc2 : 512 * (sc2 + 1)]
                            if with_bqk:
                                if sc2 == 0:
                                    nc.scalar.activation(
                                        dst, ps, AF.Identity,
                                        bias=b_sb[:, m : m + 1],
                                    )
                                else:
                                    nc.vector.tensor_scalar_add(
                                        dst, ps, b_sb[:, m : m + 1],
                                    )
                            else:
                                if sc2 == 0:
                                    nc.scalar.activation(dst, ps, AF.Copy)
                                else:
                                    nc.vector.tensor_copy(dst, ps)
                    else:
                        st = 4 * sc2 + gi - 4
                        if half == 0:
                            ps = ps_qkv.tile([128, DCOLS], F32, tag="ps_qkv",
                                             name=f"ps_v_{sc2}_{gi}")
                            group_ps[(sc2, gi)] = ps
                        else:
                            ps = group_ps.pop((sc2, gi))
                        for ct in range(4 * half, 4 * half + 4):
                            if sc2 == 0:
                                nc.tensor.matmul(
                                    ps,
                                    lhsT=xTb_[:, ct,
                                              128 * st : 128 * (st + 1)],
                                    rhs=wvb_sb[:, ct, :],
                                    start=(ct == 0),
                                    stop=(ct == N_CT - 1),
                                )
                            else:
                                nc.tensor.matmul(
                                    ps,
                                    lhsT=x8_[:, ct,
                                             128 * st : 128 * (st + 1)],
                                    rhs=wv8_sb[:, ct, :],
                                    start=(ct == 0),
                                    stop=(ct == N_CT - 1),
                                )
                        if half == 1:
                            src = ps.rearrange("p (h d) -> p h d",
                                               h=H_PER_CORE)
                            if sc2 == 0:
                                # bf16 v0 from PSUM (undo the 8x weight
                                # scale), then gpsimd dups it into fp8 v8
                                dst0 = v0_[:, st, :, 0:D]
                                nc.scalar.activation(dst0, src, AF.Copy,
                                                     scale=1.0 / W_SCALE)
                                nc.gpsimd.tensor_copy(
                                    v8_[:, st, :, 0:D], dst0
                                )
                            else:
                                dst = v8_[:, st, :, 0:D]
                                if gi % 2 == 0:
                                    nc.scalar.activation(
                                        dst, src, AF.Copy,
                                        scale=1.0 / W_SCALE)
                                else:
                                    nc.vector.tensor_scalar_mul(
                                        dst, src, 1.0 / W_SCALE
                                    )

                def is_schra(qc, j):
                    # odd tiles -> DVE Schraudolph (masked diag entries
                    # saturate to 0x80 = -0.0 via BIG_SAT), even tiles and
                    # all of qc0 -> ACT true exp (-1e9 underflows to +0)
                    return qc > 0 and j % 2 == 1

                def emit_scores(qc, hp, j):
                    t = j - 4 * qc
                    ws, N = (0, 512) if t < 0 else DIAG_WIN[t]
                    q0 = QCHUNK * qc + ws
                    diag = t >= 0 and "mask" not in ablate
                    ps_s = ps_sc.tile([128, 2, 512], F32, tag="ps_sc",
                                      name=f"ps_sc_{qc}_{hp}_{j}")
                    for u in range(2):
                        nc.tensor.matmul(
                            ps_s[:, u, 0:N],
                            lhsT=kT_sb[64 * u : 64 * (u + 1), hp,
                                       128 * j : 128 * (j + 1)],
                            rhs=qT_sb[64 * u : 64 * (u + 1), hp, q0 : q0 + N],
                            start=True,
                            stop=not diag,
                            tile_position=(64 * u, 0),
                        )
                    if diag:
                        # causal mask on the PE: the diagonal 128-block gets
                        # scores[k,q] += -BIG * Ustrict[q,k], both heads in
                        # one 256-col pass
                        nc.tensor.matmul(
                            ps_s[:, :, 0:128],
                            lhsT=ut, rhs=(bst2 if is_schra(qc, j) else bi2),
                            start=False, stop=True,
                            tile_position=(0, 0),
                        )
                    return ps_s, ws, N, t

                def emit_finalize(fqc, fhp, fpo):
                    # Copy each po tile (with its rowsum row D) to SBUF bf16
                    # immediately -- this frees the two po PSUM banks for the
                    # next chunk's PV accumulation.  The 1/rowsum scale then
                    # runs entirely on SBUF tiles, off the critical path: the
                    # rowsum rows bounce through DRAM into a [128, 8] column
                    # layout (parallel DVE reciprocal), bounce back, and are
                    # broadcast-read (stride-0 partition dim; DRAM sources
                    # only).
                    slot = 2 * fqc + fhp
                    ots = []
                    for u in range(2):
                        ot = outT_pool.tile([D + 1, QCHUNK], BF16, tag="ot",
                                            name=f"ot_{fqc}_{fhp}_{u}")
                        if u == 0:
                            nc.scalar.activation(ot, fpo[u], AF.Copy)
                        else:
                            nc.vector.tensor_copy(ot, fpo[u])
                        ots.append(ot)
                        nc.sync.dma_start(
                            out=rrd[slot, u, :], in_=ot[D : D + 1, :]
                        )
                    rc = rsum_pool.tile([128, 8], BF16, tag="rc",
                                        name=f"rc_{fqc}_{fhp}")
                    rc2 = rsum_pool.tile([128, 8], BF16, tag="rc2",
                                         name=f"rc2_{fqc}_{fhp}")
                    nc.sync.dma_start(
                        out=rc,
                        in_=bass.AP(rrd.tensor, slot * 2 * QCHUNK,
                                    [[8, 128], [1, 8]]),
                    )
                    with nc.allow_low_precision(reason="bf16 softmax scale"):
                        nc.vector.reciprocal(rc2, rc)
                    nc.sync.dma_start(
                        out=bass.AP(rrd2.tensor, slot * 2 * QCHUNK,
                                    [[8, 128], [1, 8]]),
                        in_=rc2,
                    )
                    for u in range(2):
                        h = 2 * fhp + u
                        bc = outT_pool.tile([D, QCHUNK], BF16, tag="bc",
                                            name=f"bc_{fqc}_{fhp}_{u}")
                        nc.sync.dma_start(
                            out=bc,
                            in_=bass.AP(rrd2.tensor,
                                        (slot * 2 + u) * QCHUNK,
                                        [[0, D], [1, QCHUNK]]),
                        )
                        yt = outT_pool.tile([D, QCHUNK], BF16, tag="yt",
                                            name=f"yt_{fqc}_{fhp}_{u}")
                        nc.gpsimd.tensor_mul(yt, ots[u][0:D, :], bc)
                        if with_bv:
                            nc.gpsimd.tensor_scalar_add(
                                yt, yt, bv_sbT[:, h : h + 1]
                            )
                        nc.sync.dma_start(
                            out=yT[D * h : D * (h + 1),
                                   QCHUNK * fqc : QCHUNK * (fqc + 1)],
                            in_=yt,
                        )

                # weights are rep-invariant: loaded once before rep 0.
                # x chunks 0/1 for rep r>0 were prefetched during rep r-1.
                if _rep == 0:
                    wqb_r = wqb.rearrange("(ct p) o -> p ct o", p=128)
                    xb_r = xTb.rearrange("(ct p) s -> p ct s", p=128)
                    nc.sync.dma_start(out=wqb_sb[:, 0:4, :], in_=wqb_r[:, 0:4, :])
                    nc.sync.dma_start(out=xTb_sb[:, 0:4, :], in_=xb_r[:, 0:4, :])
                    nc.sync.dma_start(out=wqb_sb[:, 4:8, :], in_=wqb_r[:, 4:8, :])
                    nc.sync.dma_start(out=xTb_sb[:, 4:8, :], in_=xb_r[:, 4:8, :])
                    nc.sync.dma_start(
                        out=wkb_sb, in_=wkb.rearrange("(ct p) o -> p ct o", p=128)
                    )
                    nc.sync.dma_start(
                        out=wvb_sb, in_=wvb.rearrange("(ct p) o -> p ct o", p=128)
                    )
                    nc.sync.dma_start(
                        out=wq8_sb, in_=wq8.rearrange("(ct p) o -> p ct o", p=128)
                    )
                    nc.sync.dma_start(
                        out=wk8_sb, in_=wk8.rearrange("(ct p) o -> p ct o", p=128)
                    )
                    nc.sync.dma_start(
                        out=wv8_sb, in_=wv8.rearrange("(ct p) o -> p ct o", p=128)
                    )
                    if with_bqk:
                        nc.sync.dma_start(out=bq_sb, in_=bq.rearrange("(m p) -> p m", p=128))
                        nc.sync.dma_start(out=bk_sb, in_=bk.rearrange("(m p) -> p m", p=128))
                    if with_bv:
                        nc.sync.dma_start(
                            out=bv_sbT,
                            in_=bv.rearrange("(h d) -> d h", h=H_PER_CORE),
                        )
                    if N_QC > 1:
                        emit_x8_dma(1)
                    # rep 0's s-chunk 0 projections run up front; for later
                    # reps they were interleaved into rep r-1's last q-chunk
                    for gi in range(8):
                        emit_qkv_half(0, gi, 0)
                        emit_qkv_half(0, gi, 1)

                xTb_next = xTb_sb2[(_rep + 1) % 2]
                x8_next = x8_sb2[(_rep + 1) % 2]
                next_bufs = (
                    xTb_next, x8_next,
                    qT_sb2[(_rep + 1) % 2], kT_sb2[(_rep + 1) % 2],
                    v8_sb2[(_rep + 1) % 2], v0_sb2[(_rep + 1) % 2],
                )
                def qc_preamble(qc):
                    if qc + 2 < N_QC:
                        emit_x8_dma(qc + 2)
                    elif _rep + 1 < reps:
                        # prefetch next rep's bf16 chunk 0 / fp8 chunk 1 into
                        # the other buffer (fp8 chunk 0 is never read)
                        sc2 = qc - 2
                        if sc2 == 0:
                            nc.sync.dma_start(
                                out=xTb_next,
                                in_=xTb.rearrange("(ct p) s -> p ct s", p=128),
                            )
                        else:
                            nc.sync.dma_start(
                                out=x8_next[:, :, 512 * sc2 : 512 * (sc2 + 1)],
                                in_=x8[:, 512 * sc2 : 512 * (sc2 + 1)].rearrange(
                                    "(ct p) s -> p ct s", p=128
                                ),
                            )
                    if qc + 1 < N_QC:
                        return [(qc + 1, gi, h, None)
                                for gi in range(8) for h in range(2)]
                    if _rep + 1 < reps:
                        # next rep's chunk-0 projections fill qc=3's bubbles
                        return [(0, gi, h, next_bufs)
                                for gi in range(8) for h in range(2)]
                    return []

                po_by = {}
                extiles = {}
                pvq = []
                interleave = []
                ilen0 = islot = n_slots = 0

                def emit_pv(entry, last):
                    fqc, fhp, fkey, fex, fws, fN = entry
                    fpo = po_by[(fqc, fhp)]
                    if fqc == 0:
                        for u in range(2):
                            nc.tensor.matmul(
                                fpo[u][:, fws : fws + fN],
                                lhsT=v0_sb[:, fkey, 2 * fhp + u, :],
                                rhs=fex[:, u, 0:fN],
                                start=(fkey == 0),
                                stop=last,
                            )
                    else:
                        for u in range(2):
                            nc.tensor.matmul(
                                fpo[u][:, fws : fws + fN],
                                lhsT=v8_sb[:, 2 * fkey : 2 * fkey + 2,
                                           2 * fhp + u, 0 : D + 1],
                                rhs=fex[:, :, u, fws : fws + fN],
                                start=(fkey == 0),
                                stop=last,
                                perf_mode=DRM,
                            )

                all_steps = [
                    (qc, hp, j)
                    for qc in range(N_QC if "attn" not in ablate else 0)
                    for hp in range(2)
                    for j in range(4 * qc + 4)
                ]
                pipeline = [emit_scores(0, 0, 0)] if all_steps else []
                next_s = 1
                for si, (qc, hp, j) in enumerate(all_steps):
                    jmax = 4 * qc + 4
                    if hp == 0 and j == 0:
                        interleave = qc_preamble(qc)
                        ilen0 = len(interleave)
                        islot = 0
                        n_slots = jmax * 2
                    if j == 0:
                        po_by[(qc, hp)] = [
                            ps_po.tile([D + 1, QCHUNK], F32, tag="ps_po",
                                       name=f"po_{qc}_{hp}_{u}")
                            for u in range(2)
                        ]
                    # the previous section's delayed last pair must land
                    # before its finalize is emitted below
                    while pvq and (pvq[0][0], pvq[0][1]) != (qc, hp):
                        emit_pv(pvq.pop(0), last=True)
                    ps_s, ws, N, t = pipeline.pop(0)
                    if qc == 0:
                        ex = exp_pool.tile([128, 2, 512], BF16, tag="ex",
                                           name=f"ex_{qc}_{hp}_{j}")
                        nc.scalar.activation(
                            ex[:, :, 0:N], ps_s[:, :, 0:N], AF.Exp,
                            scale=EXP_SCALE,
                        )
                    else:
                        pr = j // 2
                        if j % 2 == 0:
                            extiles[(hp, pr)] = exp_pool.tile(
                                [128, 2, 2, 512], F8, tag="ex",
                                name=f"ex_{qc}_{hp}_{pr}",
                            )
                        ex = extiles[(hp, pr)]
                        slab = ex[:, j % 2, :, ws : ws + N]
                        # engine split: DVE Schraudolph on odd tiles (masked
                        # entries saturate to -0.0), ACT true exp elsewhere
                        if not is_schra(qc, j):
                            nc.scalar.activation(
                                slab, ps_s[:, :, 0:N], AF.Exp,
                                scale=EXP_SCALE,
                            )
                        else:
                            nc.vector.tensor_scalar(
                                slab.bitcast(I8),
                                ps_s[:, :, 0:N],
                                SCHRA_A8, SCHRA_B8,
                                mybir.AluOpType.mult,
                                mybir.AluOpType.add,
                            )
                    # queue this step's PV pair, then emit in readiness
                    # order: delayed PV (oldest exp dep) first, finalize +
                    # fillers, and the NEXT scores last -- it waits on the
                    # freshest exp (ps_sc buffer rotation), so anything
                    # behind it in the PE FIFO would stall with it
                    if qc == 0:
                        pvq.append((qc, hp, j, ex, ws, N))
                    else:
                        if t >= 0:
                            # zero the pair-window pad left of this tile's
                            # own window
                            pws, pN = PAIR_WIN[t // 2]
                            if ws > pws:
                                nc.gpsimd.memset(
                                    ex[:, j % 2, :, pws:ws], 0.0
                                )
                        if j % 2 == 1:
                            if t >= 0:
                                pws, pN = PAIR_WIN[t // 2]
                            else:
                                pws, pN = 0, 512
                            pvq.append((qc, hp, pr, ex, pws, pN))
                            del extiles[(hp, pr)]
                    if len(pvq) > 1 and (pvq[0][0], pvq[0][1]) == (qc, hp):
                        emit_pv(pvq.pop(0), last=False)
                    if j == 1 and pending_finalize:
                        emit_finalize(*pending_finalize.pop(0))
                    islot += 1
                    while interleave and len(interleave) > (
                        ilen0 * (n_slots - islot) // n_slots
                    ):
                        emit_qkv_half(*interleave.pop(0))
                    # emit the next scores; at pair completions emit TWO
                    # adjacent score pairs so their 64-row drains overlap
                    # each other instead of the next full-width MM
                    extra = 1 if (qc > 0 and j % 2 == 1) else 0
                    while next_s < len(all_steps) and next_s <= si + 1 + extra:
                        pipeline.append(emit_scores(*all_steps[next_s]))
                        next_s += 1
                    if j == jmax - 1 and "finalize" not in ablate:
                        pending_finalize.append((qc, hp, po_by[(qc, hp)]))
                while pvq:
                    emit_pv(pvq.pop(0), last=True)
                while interleave:
                    emit_qkv_half(*interleave.pop(0))
                while pending_finalize:
                    emit_finalize(*pending_finalize.pop(0))
    legalize_sync(nc)
    return nc


_CACHE = {}


def get_program(with_bqk: bool, with_bv: bool, reps: int = 1):
    key = (with_bqk, with_bv, reps)
    if key not in _CACHE:
        _CACHE[key] = build_program(with_bqk, with_bv, reps)
    return _CACHE[key]


def make_in_maps(x, Wqk, bqk, Wv, bv):
    x = np.asarray(x, dtype=np.float32)
    Wqk = np.asarray(Wqk, dtype=np.float32)
    bqk = np.asarray(bqk, dtype=np.float32)
    Wv = np.asarray(Wv, dtype=np.float32)
    bv = np.asarray(bv, dtype=np.float32)
    xT = [np.ascontiguousarray(x[b].T) for b in range(x.shape[0])]
    in_maps = []
    for c in range(N_CORES):
        b, g = divmod(c, 4)
        cols = slice(DCOLS * g, DCOLS * (g + 1))
        wq = np.ascontiguousarray(Wqk[:, :C][:, cols]) * W_SCALE
        wk = np.ascontiguousarray(Wqk[:, C:][:, cols]) * W_SCALE
        wv = np.ascontiguousarray(Wv[:, cols]) * W_SCALE
        in_maps.append(
            {
                "xTb": xT[b][:, 0:QCHUNK].astype(NP_BF16),
                "x8": xT[b].astype(NP_F8),
                "wqb": wq.astype(NP_BF16),
                "wkb": wk.astype(NP_BF16),
                "wvb": wv.astype(NP_BF16),
                "wq8": wq.astype(NP_F8),
                "wk8": wk.astype(NP_F8),
                "wv8": wv.astype(NP_F8),
                "bq": np.ascontiguousarray(bqk[:C][cols]) * W_SCALE,
                "bk": np.ascontiguousarray(bqk[C:][cols]) * W_SCALE,
                "bv": np.ascontiguousarray(bv[cols]),
            }
        )
    return in_maps


def assemble_output(results, B):
    y = np.empty((B, S, C), dtype=np.float32)
    for c in range(N_CORES):
        b, g = divmod(c, 4)
        y[b, :, DCOLS * g : DCOLS * (g + 1)] = (
            np.asarray(results[c]["yT"]).astype(np.float32).T
        )
    return y


def kernel(x, Wqk, bqk, Wv, bv):
    in_maps = make_in_maps(x, Wqk, bqk, Wv, bv)
    with_bqk = bool(np.any(np.asarray(bqk) != 0))
    with_bv = bool(np.any(np.asarray(bv) != 0))
    nc = get_program(with_bqk, with_bv)
    res = run_bass_kernel_spmd(nc, in_maps, core_ids=list(range(N_CORES)))
    return assemble_output(res.results, np.asarray(x).shape[0])


if __name__ == "__main__":
    rng = np.random.default_rng(0)
    x = rng.standard_normal((2, S, C), dtype=np.float32)
    Wqk = rng.standard_normal((C, 2 * C), dtype=np.float32) * 0.02
    bqk = np.zeros((2 * C,), dtype=np.float32)
    Wv = rng.standard_normal((C, C), dtype=np.float32) * 0.02
    bv = np.zeros((C,), dtype=np.float32)
    out = kernel(x, Wqk, bqk, Wv, bv)
    print("kernel output:", out.shape, out.dtype, float(np.abs(out).max()))


# revision 42
# speedup vs baseline: 1.1313x; 1.0357x over previous
"""Multi-head causal self-attention (B=2, S=2048, C=1024, H=16) on 8 TRN2
NeuronCores.

Sharding: data-parallel over batch x tensor-parallel over heads.  Core c
handles batch b = c//4 and the 4 heads g = c%4 -> heads [4g, 4g+4).  Each core
computes its QKV projections from the full x[b] (weights column-sharded
head-wise), runs causal attention for its 4 heads, and writes a [S, 256]
output shard.  No cross-device communication.

Numerics: mixed bf16/fp8.  fp8e4 (TRN E4M3, max 240) matmuls with
perf_mode=DoubleRow stream 2 contraction-tiles per pass (measured 216ns/MM at
N=512 - exactly 2x bf16 throughput, LDWEIGHTS fully hidden).  fp8 error
averages out over diffuse softmax rows (q >= 512 has >= 513-way softmax, max
weight ~1%), so everything fp8-able for those rows is fp8:
  - q/k projections for s-chunks 1-3: fp8-DR (inputs x8 = fp8(x^T),
    wq8/wk8 = fp8(8*W); the 8x weight scale keeps W ~N(0,0.02) out of e4m3's
    subnormal range; exp scale absorbs the 64x on scores).
  - v projection st>=4 (k >= 512): fp8 operands (bf16-rate non-DR matmuls),
    v stored fp8 with a ones column (PV row sums) padded to 68 cols so the
    DR k-pair stride is 16B-aligned.
  - PV for q-chunks >= 1: fp8-DR over k-block PAIRS.  exp writes fp8 into
    pair tiles ex[128, 2(jpair), 2(head), 512]; diagonal windows are
    zero-padded (gpsimd memset) so a pair can stream its union window.
Early rows stay accurate on the bf16 path: chunk-0 q/k/v projections are
bf16 (from xTb/w*b), q-chunk 0 attention is the baseline bf16 path with a
bf16 copy of v for k < 512 (v0), protecting rows whose softmax is
concentrated (q < ~128: out ~= v directly, needs better than fp8).

softmax: exp((s)*0.125/64) straight out of PSUM; no max subtraction (scaled
scores are O(+-2), 17 sigma from e4m3's inf at 240 and from Schraudolph's
int8 sign flip).  Causal masking is folded into the scores PSUM on the PE:
diagonal tiles get ONE extra accumulating matmul covering both heads,
scores[k,q] += -BIG * Ustrict[q,k] (lhsT=Ustrict, rhs=-BIG*I), so no
elementwise mask op exists at all.  BIG is 1e9 for true-exp (ACT) tiles
(exp underflows to +0) and 16384 for Schraudolph (DVE) tiles: the int8
conversion saturates at -128 = 0x80 = -0.0 in e4m3 (measured), so masked
weights are exactly -0.0.

Engine split: PE does all matmuls incl. masking; ACT does true exp (fp8/bf16
out) and half the finalize copies; DVE does Schraudolph fast-exp (int8
bit-trick into fp8) for the odd off-diagonal tiles plus q/k/v PSUM->SBUF
copies and the reciprocal; gpsimd (slow, SBUF-only) does pair-pad memsets,
v0->fp8 dup casts and the finalize 1/rowsum muls.

Scheduling: emission is software-pipelined -- scores run one k-tile ahead of
exp, PV-DR lags one PAIR behind its exp (so the PE never stalls on a fresh
exp), score pairs are emitted in runs of two so their 64-row drains overlap
each other instead of the next full-width matmul, the next s-chunk's
projection groups are interleaved at half-chain granularity evenly across
the j loop, and rep r+1's chunk-0 projections fill rep r's last q-chunk.
The whole rep is one flat (qc, hp, j) pipeline -- scores/exp/PV flow across
section boundaries; within each step instructions are emitted in readiness
order (delayed PV, finalize, projection fillers, then the freshest scores
last) because engine queues are strict FIFO and one stalled instruction
blocks everything behind it.  Finalizes are deferred past the next
section's lead-in.

This walrus build only accepts sem waits on EventSemaphore instructions (and
one update on non-DMA instructions), so legalize_sync() post-processes the
Tile-scheduled BIR to hoist waits / split updates, and TileContextPatched
replaces the stock drain-with-eq-wait tail barrier.
"""

import os
import sys

for _p in ("/opt/trn_rl_repo",):
    if _p not in sys.path and os.path.isdir(_p):
        sys.path.append(_p)

import ml_dtypes
import numpy as np

import bass_rust
import concourse.bass as bass
import concourse.mybir as mybir
import concourse.tile as tile
from concourse import library_config
from concourse.bass_utils import run_bass_kernel_spmd
from concourse.masks import make_identity
from concourse.tile import ScopedClock

F32 = mybir.dt.float32
BF16 = mybir.dt.bfloat16
F8 = mybir.dt.float8e4
I8 = mybir.dt.int8
AF = mybir.ActivationFunctionType
DRM = mybir.MatmulPerfMode.DoubleRow
NP_BF16 = ml_dtypes.bfloat16
NP_F8 = ml_dtypes.float8_e4m3


class TileContextPatched(tile.TileContext):
    """Works around this walrus build's 1-sync-wait-per-instruction limit on
    Drain (and the Drain-with-eq-wait barrier form): the tail drain's
    vector-clock waits are re-emitted as individual wait_ge instructions, and
    the engine quiesce/semaphore-reset is done with plain ge-wait semaphores.
    """

    def _drain_and_barrier(self, tick_clock, wait_clock):
        nc = self.nc
        drain = nc.sync.drain()
        wait_clock.add_sem_waits(
            drain.ins, ScopedClock({None: tick_clock.global_clock})
        )
        waits = list(drain.ins.sync_info.on_wait)
        drain.ins.sync_info.on_wait = []
        by_name = {}
        for _k, h in self.sems.allocated().items():
            by_name[getattr(h, "name", str(_k))] = h
        for w in waits:
            h = by_name.get(w.ant_name)
            assert h is not None, f"no handle for sem {w.ant_name}"
            nc.sync.wait_ge(h, w.wait_value)

        done = nc.alloc_semaphore("tile_tail_done")
        go = nc.alloc_semaphore("tile_tail_go")
        n_other = 0
        for _et, eng in nc.engines.items():
            if eng is nc.sync:
                continue
            eng.nop(nofuse=True, hint="tail_done").then_inc(done, 1)
            n_other += 1
        nc.sync.wait_ge(done, n_other)
        nc.sync.nop(nofuse=True, hint="tail_go").then_inc(go, 1)
        nc.gpsimd.wait_ge(go, 1)

        popped = nc._tile_sem_poison_stack.pop()
        assert popped is self._sem_poison
        nc.clear_and_free_semaphores(
            list(self.sems.allocated().values()) + [done, go]
        )


def legalize_sync(nc):
    """Rewrite sync_info to this walrus build's per-instruction limits:
    compute/DMA instructions carry NO waits (hoisted onto preceding
    EventSemaphore instrs, <=2 ge-waits each); non-DMA instructions carry at
    most 1 update (extras move to following EventSemaphores, 1 each, which
    retire only after the preceding same-engine instruction completes).
    DMA updates are never moved (they fire at transfer completion)."""
    for f in nc.m.functions:
        for b in f.blocks:
            changed = False
            new = []
            for inst in b.instructions:
                si = getattr(inst, "sync_info", None)
                if si is None:
                    new.append(inst)
                    continue
                waits = list(si.on_wait)
                upds = list(si.on_update)
                opcode = getattr(inst, "opcode", "") or ""
                is_ev = opcode == "EventSemaphore"
                is_dma = "DMA" in opcode
                max_w = 2 if is_ev else 0
                hoist = []
                if len(waits) > max_w:
                    hoist = waits[max_w:]
                    waits = waits[:max_w]
                extra_upd = []
                max_u = 1 if not is_dma else len(upds)
                if len(upds) > max_u:
                    extra_upd = upds[max_u:]
                    upds = upds[:max_u]
                if hoist or extra_upd:
                    changed = True
                    for i in range(0, len(hoist), 2):
                        ev = mybir.InstEventSemaphore(
                            name=f"evw-{nc.next_id()}", ins=[], outs=[]
                        )
                        ev.engine = inst.engine
                        ev.sync_info = bass_rust.SyncInfo(
                            on_update=[], on_wait=hoist[i : i + 2]
                        )
                        nc.register_instruction(ev)
                        new.append(ev)
                    inst.sync_info = bass_rust.SyncInfo(
                        on_update=upds, on_wait=waits
                    )
                    new.append(inst)
                    for u in extra_upd:
                        ev = mybir.InstEventSemaphore(
                            name=f"evu-{nc.next_id()}", ins=[], outs=[]
                        )
                        ev.engine = inst.engine
                        ev.sync_info = bass_rust.SyncInfo(
                            on_update=[u], on_wait=[]
                        )
                        nc.register_instruction(ev)
                        new.append(ev)
                else:
                    new.append(inst)
            if changed:
                b.instructions = new


# ----------------------------------------------------------------------------
# Problem constants (hardcoded per contest rules)
S = 2048          # sequence length
C = 1024          # embed / qk channels
H_PER_CORE = 4    # heads per core (16 heads / 8 cores * 2 batch-replicas)
D = 64            # head dim
DP = 68           # fp8 v row padded so the DR k-pair stride (4*DP) is %16
DCOLS = H_PER_CORE * D            # 256 weight columns per core
N_CT = C // 128                   # 8 contraction tiles for projections
N_SB = S // 128                   # 16 sequence blocks of 128
QCHUNK = 512
N_QC = S // QCHUNK                # 4 q chunks
N_CORES = 8

W_SCALE = 8.0                     # host-side q/k/v weight scale (fp8 range)
EXP_SCALE = 0.125 / (W_SCALE * W_SCALE)   # folded softmax scale

# Schraudolph fast-exp in fp8e4 bit-space: bits8(exp(y)) ~= 8/ln2 * y +
# (7*8 - 0.35) for y = EXP_SCALE*s.  One DVE tensor_scalar (mult, add) with
# int8 output writes exp directly into the fp8 ex tile.  Max rel err ~7%;
# applied only to off-diagonal tiles of diffuse rows (q >= 512).
SCHRA_A8 = (8.0 / 0.6931471805599453) * EXP_SCALE
SCHRA_B8 = 7.0 * 8.0 - 0.35
BIG_EXACT = 1e9       # mask offset for ACT tiles: exp underflows to +0
# mask offset for DVE Schraudolph tiles: bits = A8*(s-BIG_SAT)+B8 <= -283
# for any score, and the DVE float->int8 conversion SATURATES at -128 =
# 0x80 = -0.0 in e4m3 -> masked weights are exactly -0.0 (measured).
BIG_SAT = 16384.0

DIAG_WIN = {0: (0, 512), 1: (128, 384), 2: (256, 256), 3: (384, 128)}
# DR pair windows for the two diagonal pairs (t in {0,1} and {2,3}):
# union of the member windows
PAIR_WIN = {0: (0, 512), 1: (256, 256)}


def build_program(with_bqk: bool, with_bv: bool, reps: int = 1, ablate=()):
    nc = bass.Bass("TRN2", target_bir_lowering=False, debug=False)

    # bf16 x^T, only s-chunk 0 (feeds the bf16 chunk-0 projections)
    xTb = nc.dram_tensor("xTb", [C, QCHUNK], BF16, kind="ExternalInput").ap()
    # fp8 x^T, full (feeds fp8-DR q/k projections of chunks 1-3 and fp8
    # v projections of st 4-15)
    x8 = nc.dram_tensor("x8", [C, S], F8, kind="ExternalInput").ap()
    wqb = nc.dram_tensor("wqb", [C, DCOLS], BF16, kind="ExternalInput").ap()
    wkb = nc.dram_tensor("wkb", [C, DCOLS], BF16, kind="ExternalInput").ap()
    wvb = nc.dram_tensor("wvb", [C, DCOLS], BF16, kind="ExternalInput").ap()
    wq8 = nc.dram_tensor("wq8", [C, DCOLS], F8, kind="ExternalInput").ap()
    wk8 = nc.dram_tensor("wk8", [C, DCOLS], F8, kind="ExternalInput").ap()
    wv8 = nc.dram_tensor("wv8", [C, DCOLS], F8, kind="ExternalInput").ap()
    bq = nc.dram_tensor("bq", [DCOLS], F32, kind="ExternalInput").ap()
    bk = nc.dram_tensor("bk", [DCOLS], F32, kind="ExternalInput").ap()
    bv = nc.dram_tensor("bv", [DCOLS], F32, kind="ExternalInput").ap()
    # output is produced TRANSPOSED ([DCOLS, S]); the host re-transposes when
    # assembling the full [B, S, C] result (pure layout glue, no math)
    yT = nc.dram_tensor("yT", [DCOLS, S], BF16, kind="ExternalOutput").ap()
    # DRAM bounce for the 1/rowsum row: DMA reads from DRAM may broadcast
    # (stride-0 partition dim), SBUF sources may not
    rrd = nc.dram_tensor("rrd", [8, 2, QCHUNK], BF16, kind="Internal").ap()
    rrd2 = nc.dram_tensor("rrd2", [8, 2, QCHUNK], BF16, kind="Internal").ap()

    with TileContextPatched(nc) as tc:
        with (
            tc.tile_pool(name="singles", bufs=1) as singles,
            tc.tile_pool(name="exp", bufs=12) as exp_pool,
            tc.tile_pool(name="outT", bufs=10) as outT_pool,
            tc.tile_pool(name="rsum", bufs=16) as rsum_pool,
            tc.tile_pool(name="ps_qkv", bufs=2, space="PSUM") as ps_qkv,
            tc.tile_pool(name="ps_sc", bufs=2, space="PSUM") as ps_sc,
            tc.tile_pool(name="ps_po", bufs=2, space="PSUM") as ps_po,
        ):
            # ---- persistent SBUF tensors -----------------------------------
            # x/q/k/v are double-buffered by rep parity so rep r+1's
            # projections overlap rep r's attention tail (the marginal-rep
            # cost is what the harness measures).
            xTb_sb2 = [singles.tile([128, N_CT, QCHUNK], BF16,
                                    name=f"xTb_sb{_i}") for _i in range(2)]
            x8_sb2 = [singles.tile([128, N_CT, S], F8, name=f"x8_sb{_i}")
                      for _i in range(2)]
            wqb_sb = singles.tile([128, N_CT, DCOLS], BF16)
            wkb_sb = singles.tile([128, N_CT, DCOLS], BF16)
            wvb_sb = singles.tile([128, N_CT, DCOLS], BF16)
            wq8_sb = singles.tile([128, N_CT, DCOLS], F8)
            wk8_sb = singles.tile([128, N_CT, DCOLS], F8)
            wv8_sb = singles.tile([128, N_CT, DCOLS], F8)
            qT_sb2 = [singles.tile([128, 2, S], BF16, name=f"qT_sb{_i}")
                      for _i in range(2)]
            kT_sb2 = [singles.tile([128, 2, S], BF16, name=f"kT_sb{_i}")
                      for _i in range(2)]
            # fp8 v (all st blocks), padded to DP cols; ones col at D
            v8_sb2 = [
                singles.tile([128, N_SB, H_PER_CORE, DP], F8,
                             name=f"v8_sb{_i}")
                for _i in range(2)
            ]
            # bf16 v for k < 512 (q-chunk 0 attention)
            v0_sb2 = [
                singles.tile([128, 4, H_PER_CORE, D + 1], BF16,
                             name=f"v0_sb{_i}")
                for _i in range(2)
            ]
            # mask constants: ut[p,f]=1 iff f>p; bi2/bst2 = -BIG*I for
            # both head slices at once (out free dims (2, 128))
            ut = singles.tile([128, 128], BF16, name="ut")
            bi2 = singles.tile([128, 2, 128], BF16, name="bi2")
            bst2 = singles.tile([128, 2, 128], BF16, name="bst2")
            bq_sb = singles.tile([128, 2], F32) if with_bqk else None
            bk_sb = singles.tile([128, 2], F32) if with_bqk else None
            bv_sbT = singles.tile([D, H_PER_CORE], F32, name="bv_sbT") if with_bv else None

            # ---- constants / masks ----------------------------------------
            nc.gpsimd.memset(ut, 1.0)
            nc.gpsimd.affine_select(
                out=ut, in_=ut, compare_op=mybir.AluOpType.is_gt,
                fill=0.0, base=0, pattern=[[1, 128]], channel_multiplier=-1,
            )
            nc.gpsimd.memset(bi2, -BIG_EXACT)
            nc.gpsimd.affine_select(
                out=bi2, in_=bi2, compare_op=mybir.AluOpType.is_equal,
                fill=0.0, base=0, pattern=[[0, 2], [1, 128]],
                channel_multiplier=-1,
            )
            nc.gpsimd.memset(bst2, -BIG_SAT)
            nc.gpsimd.affine_select(
                out=bst2, in_=bst2, compare_op=mybir.AluOpType.is_equal,
                fill=0.0, base=0, pattern=[[0, 2], [1, 128]],
                channel_multiplier=-1,
            )
            for _v in v8_sb2:
                # ones column for PV row sums; zero padding cols D+1..DP
                nc.gpsimd.memset(_v[:, :, :, D : D + 1], 1.0)
                nc.gpsimd.memset(_v[:, :, :, D + 1 : DP], 0.0)
            for _v in v0_sb2:
                nc.vector.memset(_v[:, :, :, D : D + 1], 1.0)

            for _rep in range(reps):
                xTb_sb = xTb_sb2[_rep % 2]
                x8_sb = x8_sb2[_rep % 2]
                qT_sb = qT_sb2[_rep % 2]
                kT_sb = kT_sb2[_rep % 2]
                v8_sb = v8_sb2[_rep % 2]
                v0_sb = v0_sb2[_rep % 2]
                pending_finalize = []
                group_ps = {}

                def emit_x8_dma(sc2):
                    nc.sync.dma_start(
                        out=x8_sb[:, :, 512 * sc2 : 512 * (sc2 + 1)],
                        in_=x8[:, 512 * sc2 : 512 * (sc2 + 1)].rearrange(
                            "(ct p) s -> p ct s", p=128
                        ),
                    )

                def emit_qkv_half(sc2, gi, half, bufs=None):
                    """Half of a projection group.  gi 0..3: qT/kT projection
                    (tensor gi//2, Mtile gi%2); gi 4..7: v block
                    st = 4*sc2 + gi - 4.  qT/kT are [d, s] (Mtile m = heads
                    2m, 2m+1); v is natural [s, d] with the ones column for
                    the PV row sums.  sc2 == 0 is the bf16 path; sc2 >= 1 q/k
                    are fp8-DR, v is fp8 non-DR.  bufs overrides the SBUF
                    buffer set (next-rep chunk-0 groups interleaved into this
                    rep's last q-chunk).  half 0 runs the first half of the
                    contraction chain, half 1 finishes it and emits the
                    PSUM->SBUF copy."""
                    xTb_, x8_, qT_, kT_, v8_, v0_ = (
                        bufs
                        if bufs is not None
                        else (xTb_sb, x8_sb, qT_sb, kT_sb, v8_sb, v0_sb)
                    )
                    if gi < 4:
                        m = gi % 2
                        if half == 0:
                            ps = ps_qkv.tile([128, 512], F32, tag="ps_qkv",
                                             name=f"ps_qk_{sc2}_{gi}")
                            group_ps[(sc2, gi)] = ps
                        else:
                            ps = group_ps.pop((sc2, gi))
                        if sc2 == 0:
                            w_sb, t_sb, b_sb = (
                                (wqb_sb, qT_, bq_sb), (wkb_sb, kT_, bk_sb)
                            )[gi // 2]
                            for ct in range(4 * half, 4 * half + 4):
                                nc.tensor.matmul(
                                    ps,
                                    lhsT=w_sb[:, ct, 128 * m : 128 * (m + 1)],
                                    rhs=xTb_[:, ct, :],
                                    start=(ct == 0),
                                    stop=(ct == N_CT - 1),
                                )
                        else:
                            w_sb, t_sb, b_sb = (
                                (wq8_sb, qT_, bq_sb), (wk8_sb, kT_, bk_sb)
                            )[gi // 2]
                            for cp in range(2 * half, 2 * half + 2):
                                nc.tensor.matmul(
                                    ps,
                                    lhsT=w_sb[:, 2 * cp : 2 * cp + 2,
                                              128 * m : 128 * (m + 1)],
                                    rhs=x8_[:, 2 * cp : 2 * cp + 2,
                                            512 * sc2 : 512 * (sc2 + 1)],
                                    start=(cp == 0),
                                    stop=(cp == N_CT // 2 - 1),
                                    perf_mode=DRM,
                                )
                        if half == 1:
                            dst = t_sb[:, m, 512 * sc2 : 512 * (sc2 + 1)]
                            if with_bqk:
                                if sc2 == 0:
                                    nc.scalar.activation(
                                        dst, ps, AF.Identity,
                                        bias=b_sb[:, m : m + 1],
                                    )
                                else:
                                    nc.vector.tensor_scalar_add(
                                        dst, ps, b_sb[:, m : m + 1],
                                    )
                            else:
                                if sc2 == 0:
                                    nc.scalar.activation(dst, ps, AF.Copy)
                                else:
                                    nc.vector.tensor_copy(dst, ps)
                    else:
                        st = 4 * sc2 + gi - 4
                        if half == 0:
                            ps = ps_qkv.tile([128, DCOLS], F32, tag="ps_qkv",
                                             name=f"ps_v_{sc2}_{gi}")
                            group_ps[(sc2, gi)] = ps
                        else:
                            ps = group_ps.pop((sc2, gi))
                        for ct in range(4 * half, 4 * half + 4):
                            if sc2 == 0:
                                nc.tensor.matmul(
                                    ps,
                                    lhsT=xTb_[:, ct,
                                              128 * st : 128 * (st + 1)],
                                    rhs=wvb_sb[:, ct, :],
                                    start=(ct == 0),
                                    stop=(ct == N_CT - 1),
                                )
                            else:
                                nc.tensor.matmul(
                                    ps,
                                    lhsT=x8_[:, ct,
                                             128 * st : 128 * (st + 1)],
                                    rhs=wv8_sb[:, ct, :],
                                    start=(ct == 0),
                                    stop=(ct == N_CT - 1),
                                )
                        if half == 1:
                            src = ps.rearrange("p (h d) -> p h d",
                                               h=H_PER_CORE)
                            if sc2 == 0:
                                # bf16 v0 from PSUM (undo the 8x weight
                                # scale), then gpsimd dups it into fp8 v8
                                dst0 = v0_[:, st, :, 0:D]
                                nc.scalar.activation(dst0, src, AF.Copy,
                                                     scale=1.0 / W_SCALE)
                                nc.gpsimd.tensor_copy(
                                    v8_[:, st, :, 0:D], dst0
                                )
                            else:
                                dst = v8_[:, st, :, 0:D]
                                if gi % 2 == 0:
                                    nc.scalar.activation(
                                        dst, src, AF.Copy,
                                        scale=1.0 / W_SCALE)
                                else:
                                    nc.vector.tensor_scalar_mul(
                                        dst, src, 1.0 / W_SCALE
                                    )

                def is_schra(qc, j):
                    # odd tiles -> DVE Schraudolph (masked diag entries
                    # saturate to 0x80 = -0.0 via BIG_SAT), even tiles and
                    # all of qc0 -> ACT true exp (-1e9 underflows to +0)
                    return qc > 0 and j % 2 == 1

                def emit_scores(qc, hp, j):
                    t = j - 4 * qc
                    ws, N = (0, 512) if t < 0 else DIAG_WIN[t]
                    q0 = QCHUNK * qc + ws
                    diag = t >= 0 and "mask" not in ablate
                    ps_s = ps_sc.tile([128, 2, 512], F32, tag="ps_sc",
                                      name=f"ps_sc_{qc}_{hp}_{j}")
                    for u in range(2):
                        nc.tensor.matmul(
                            ps_s[:, u, 0:N],
                            lhsT=kT_sb[64 * u : 64 * (u + 1), hp,
                                       128 * j : 128 * (j + 1)],
                            rhs=qT_sb[64 * u : 64 * (u + 1), hp, q0 : q0 + N],
                            start=True,
                            stop=not diag,
                            tile_position=(64 * u, 0),
                        )
                    if diag:
                        # causal mask on the PE: the diagonal 128-block gets
                        # scores[k,q] += -BIG * Ustrict[q,k], both heads in
                        # one 256-col pass
                        nc.tensor.matmul(
                            ps_s[:, :, 0:128],
                            lhsT=ut, rhs=(bst2 if is_schra(qc, j) else bi2),
                            start=False, stop=True,
                            tile_position=(0, 0),
                        )
                    return ps_s, ws, N, t

                def emit_finalize(fqc, fhp, fpo):
                    # Copy each po tile (with its rowsum row D) to SBUF bf16
                    # immediately -- this frees the two po PSUM banks for the
                    # next chunk's PV accumulation.  The 1/rowsum scale then
                    # runs entirely on SBUF tiles, off the critical path: the
                    # rowsum rows bounce through DRAM into a [128, 8] column
                    # layout (parallel DVE reciprocal), bounce back, and are
                    # broadcast-read (stride-0 partition dim; DRAM sources
                    # only).
                    slot = 2 * fqc + fhp
                    ots = []
                    for u in range(2):
                        ot = outT_pool.tile([D + 1, QCHUNK], BF16, tag="ot",
                                            name=f"ot_{fqc}_{fhp}_{u}")
                        if u == 0:
                            nc.scalar.activation(ot, fpo[u], AF.Copy)
                        else:
                            nc.vector.tensor_copy(ot, fpo[u])
                        ots.append(ot)
                        nc.sync.dma_start(
                            out=rrd[slot, u, :], in_=ot[D : D + 1, :]
                        )
                    rc = rsum_pool.tile([128, 8], BF16, tag="rc",
                                        name=f"rc_{fqc}_{fhp}")
                    rc2 = rsum_pool.tile([128, 8], BF16, tag="rc2",
                                         name=f"rc2_{fqc}_{fhp}")
                    nc.sync.dma_start(
                        out=rc,
                        in_=bass.AP(rrd.tensor, slot * 2 * QCHUNK,
                                    [[8, 128], [1, 8]]),
                    )
                    with nc.allow_low_precision(reason="bf16 softmax scale"):
                        nc.vector.reciprocal(rc2, rc)
                    nc.sync.dma_start(
                        out=bass.AP(rrd2.tensor, slot * 2 * QCHUNK,
                                    [[8, 128], [1, 8]]),
                        in_=rc2,
                    )
                    for u in range(2):
                        h = 2 * fhp + u
                        bc = outT_pool.tile([D, QCHUNK], BF16, tag="bc",
                                            name=f"bc_{fqc}_{fhp}_{u}")
                        nc.sync.dma_start(
                            out=bc,
                            in_=bass.AP(rrd2.tensor,
                                        (slot * 2 + u) * QCHUNK,
                                        [[0, D], [1, QCHUNK]]),
                        )
                        yt = outT_pool.tile([D, QCHUNK], BF16, tag="yt",
                                            name=f"yt_{fqc}_{fhp}_{u}")
                        nc.gpsimd.tensor_mul(yt, ots[u][0:D, :], bc)
                        if with_bv:
                            nc.gpsimd.tensor_scalar_add(
                                yt, yt, bv_sbT[:, h : h + 1]
                            )
                        nc.sync.dma_start(
                            out=yT[D * h : D * (h + 1),
                                   QCHUNK * fqc : QCHUNK * (fqc + 1)],
                            in_=yt,
                        )

                # weights are rep-invariant: loaded once before rep 0.
                # x chunks 0/1 for rep r>0 were prefetched during rep r-1.
                if _rep == 0:
                    wqb_r = wqb.rearrange("(ct p) o -> p ct o", p=128)
                    xb_r = xTb.rearrange("(ct p) s -> p ct s", p=128)
                    nc.sync.dma_start(out=wqb_sb[:, 0:4, :], in_=wqb_r[:, 0:4, :])
                    nc.sync.dma_start(out=xTb_sb[:, 0:4, :], in_=xb_r[:, 0:4, :])
                    nc.sync.dma_start(out=wqb_sb[:, 4:8, :], in_=wqb_r[:, 4:8, :])
                    nc.sync.dma_start(out=xTb_sb[:, 4:8, :], in_=xb_r[:, 4:8, :])
                    nc.sync.dma_start(
                        out=wkb_sb, in_=wkb.rearrange("(ct p) o -> p ct o", p=128)
                    )
                    nc.sync.dma_start(
                        out=wvb_sb, in_=wvb.rearrange("(ct p) o -> p ct o", p=128)
                    )
                    nc.sync.dma_start(
                        out=wq8_sb, in_=wq8.rearrange("(ct p) o -> p ct o", p=128)
                    )
                    nc.sync.dma_start(
                        out=wk8_sb, in_=wk8.rearrange("(ct p) o -> p ct o", p=128)
                    )
                    nc.sync.dma_start(
                        out=wv8_sb, in_=wv8.rearrange("(ct p) o -> p ct o", p=128)
                    )
                    if with_bqk:
                        nc.sync.dma_start(out=bq_sb, in_=bq.rearrange("(m p) -> p m", p=128))
                        nc.sync.dma_start(out=bk_sb, in_=bk.rearrange("(m p) -> p m", p=128))
                    if with_bv:
                        nc.sync.dma_start(
                            out=bv_sbT,
                            in_=bv.rearrange("(h d) -> d h", h=H_PER_CORE),
                        )
                    if N_QC > 1:
                        emit_x8_dma(1)
                    # rep 0's s-chunk 0 projections run up front; for later
                    # reps they were interleaved into rep r-1's last q-chunk
                    for gi in range(8):
                        emit_qkv_half(0, gi, 0)
                        emit_qkv_half(0, gi, 1)

                xTb_next = xTb_sb2[(_rep + 1) % 2]
                x8_next = x8_sb2[(_rep + 1) % 2]
                next_bufs = (
                    xTb_next, x8_next,
                    qT_sb2[(_rep + 1) % 2], kT_sb2[(_rep + 1) % 2],
                    v8_sb2[(_rep + 1) % 2], v0_sb2[(_rep + 1) % 2],
                )
                def qc_preamble(qc):
                    if qc + 2 < N_QC:
                        emit_x8_dma(qc + 2)
                    elif _rep + 1 < reps:
                        # prefetch next rep's bf16 chunk 0 / fp8 chunk 1 into
                        # the other buffer (fp8 chunk 0 is never read)
                        sc2 = qc - 2
                        if sc2 == 0:
                            nc.sync.dma_start(
                                out=xTb_next,
                                in_=xTb.rearrange("(ct p) s -> p ct s", p=128),
                            )
                        else:
                            nc.sync.dma_start(
                                out=x8_next[:, :, 512 * sc2 : 512 * (sc2 + 1)],
                                in_=x8[:, 512 * sc2 : 512 * (sc2 + 1)].rearrange(
                                    "(ct p) s -> p ct s", p=128
                                ),
                            )
                    if qc + 1 < N_QC:
                        return [(qc + 1, gi, h, None)
                                for gi in range(8) for h in range(2)]
                    if _rep + 1 < reps:
                        # next rep's chunk-0 projections fill qc=3's bubbles
                        return [(0, gi, h, next_bufs)
                                for gi in range(8) for h in range(2)]
                    return []

                po_by = {}
                extiles = {}
                pvq = []
                interleave = []
                ilen0 = islot = n_slots = 0

                def emit_pv(entry, last):
                    fqc, fhp, fkey, fex, fws, fN = entry
                    fpo = po_by[(fqc, fhp)]
                    if fqc == 0:
                        for u in range(2):
                            nc.tensor.matmul(
                                fpo[u][:, fws : fws + fN],
                                lhsT=v0_sb[:, fkey, 2 * fhp + u, :],
                                rhs=fex[:, u, 0:fN],
                                start=(fkey == 0),
                                stop=last,
                            )
                    else:
                        for u in range(2):
                            nc.tensor.matmul(
                                fpo[u][:, fws : fws + fN],
                                lhsT=v8_sb[:, 2 * fkey : 2 * fkey + 2,
                                           2 * fhp + u, 0 : D + 1],
                                rhs=fex[:, :, u, fws : fws + fN],
                                start=(fkey == 0),
                                stop=last,
                                perf_mode=DRM,
                            )

                all_steps = [
                    (qc, hp, j)
                    for qc in range(N_QC if "attn" not in ablate else 0)
                    for hp in range(2)
                    for j in range(4 * qc + 4)
                ]
                pipeline = [emit_scores(0, 0, 0)] if all_steps else []
                next_s = 1
                for si, (qc, hp, j) in enumerate(all_steps):
                    jmax = 4 * qc + 4
                    if hp == 0 and j == 0:
                        interleave = qc_preamble(qc)
                        ilen0 = len(interleave)
                        islot = 0
                        n_slots = jmax * 2
                    if j == 0:
                        po_by[(qc, hp)] = [
                            ps_po.tile([D + 1, QCHUNK], F32, tag="ps_po",
                                       name=f"po_{qc}_{hp}_{u}")
                            for u in range(2)
                        ]
                    # the previous section's delayed last pair must land
                    # before its finalize is emitted below
                    while pvq and (pvq[0][0], pvq[0][1]) != (qc, hp):
                        emit_pv(pvq.pop(0), last=True)
                    ps_s, ws, N, t = pipeline.pop(0)
                    if qc == 0:
                        ex = exp_pool.tile([128, 2, 512], BF16, tag="ex",
                                           name=f"ex_{qc}_{hp}_{j}")
                        nc.scalar.activation(
                            ex[:, :, 0:N], ps_s[:, :, 0:N], AF.Exp,
                            scale=EXP_SCALE,
                        )
                    else:
                        pr = j // 2
                        if j % 2 == 0:
                            extiles[(hp, pr)] = exp_pool.tile(
                                [128, 2, 2, 512], F8, tag="ex",
                                name=f"ex_{qc}_{hp}_{pr}",
                            )
                        ex = extiles[(hp, pr)]
                        slab = ex[:, j % 2, :, ws : ws + N]
                        # engine split: DVE Schraudolph on odd tiles (masked
                        # entries saturate to -0.0), ACT true exp elsewhere
                        if not is_schra(qc, j):
                            nc.scalar.activation(
                                slab, ps_s[:, :, 0:N], AF.Exp,
                                scale=EXP_SCALE,
                            )
                        else:
                            nc.vector.tensor_scalar(
                                slab.bitcast(I8),
                                ps_s[:, :, 0:N],
                                SCHRA_A8, SCHRA_B8,
                                mybir.AluOpType.mult,
                                mybir.AluOpType.add,
                            )
                    # queue this step's PV pair, then emit in readiness
                    # order: delayed PV (oldest exp dep) first, finalize +
                    # fillers, and the NEXT scores last -- it waits on the
                    # freshest exp (ps_sc buffer rotation), so anything
                    # behind it in the PE FIFO would stall with it
                    if qc == 0:
                        pvq.append((qc, hp, j, ex, ws, N))
                    else:
                        if t >= 0:
                            # zero the pair-window pad left of this tile's
                            # own window
                            pws, pN = PAIR_WIN[t // 2]
                            if ws > pws:
                                nc.gpsimd.memset(
                                    ex[:, j % 2, :, pws:ws], 0.0
                                )
                        if j % 2 == 1:
                            if t >= 0:
                                pws, pN = PAIR_WIN[t // 2]
                            else:
                                pws, pN = 0, 512
                            pvq.append((qc, hp, pr, ex, pws, pN))
                            del extiles[(hp, pr)]
                    if len(pvq) > 1 and (pvq[0][0], pvq[0][1]) == (qc, hp):
                        emit_pv(pvq.pop(0), last=False)
                    if j == 1 and pending_finalize:
                        emit_finalize(*pending_finalize.pop(0))
                    islot += 1
                    while interleave and len(interleave) > (
                        ilen0 * (n_slots - islot) // n_slots
                    ):
                        emit_qkv_half(*interleave.pop(0))
                    # emit the next scores; at pair completions emit TWO
                    # adjacent score pairs so their 64-row drains overlap
                    # each other instead of the next full-width MM
                    extra = 1 if (qc > 0 and j % 2 == 1) else 0
                    while next_s < len(all_steps) and next_s <= si + 1 + extra:
                        pipeline.append(emit_scores(*all_steps[next_s]))
                        next_s += 1
                    if j == jmax - 1 and "finalize" not in ablate:
                        pending_finalize.append((qc, hp, po_by[(qc, hp)]))
                while pvq:
                    emit_pv(pvq.pop(0), last=True)
                while interleave:
                    emit_qkv_half(*interleave.pop(0))
                while pending_finalize:
                    emit_finalize(*pending_finalize.pop(0))
    legalize_sync(nc)
    return nc


_CACHE = {}


def get_program(with_bqk: bool, with_bv: bool, reps: int = 1):
    key = (with_bqk, with_bv, reps)
    if key not in _CACHE:
        _CACHE[key] = build_program(with_bqk, with_bv, reps)
    return _CACHE[key]


def make_in_maps(x, Wqk, bqk, Wv, bv):
    x = np.asarray(x, dtype=np.float32)
    Wqk = np.asarray(Wqk, dtype=np.float32)
    bqk = np.asarray(bqk, dtype=np.float32)
    Wv = np.asarray(Wv, dtype=np.float32)
    bv = np.asarray(bv, dtype=np.float32)
    xT = [np.ascontiguousarray(x[b].T) for b in range(x.shape[0])]
    in_maps = []
    for c in range(N_CORES):
        b, g = divmod(c, 4)
        cols = slice(DCOLS * g, DCOLS * (g + 1))
        wq = np.ascontiguousarray(Wqk[:, :C][:, cols]) * W_SCALE
        wk = np.ascontiguousarray(Wqk[:, C:][:, cols]) * W_SCALE
        wv = np.ascontiguousarray(Wv[:, cols]) * W_SCALE
        in_maps.append(
            {
                "xTb": xT[b][:, 0:QCHUNK].astype(NP_BF16),
                "x8": xT[b].astype(NP_F8),
                "wqb": wq.astype(NP_BF16),
                "wkb": wk.astype(NP_BF16),
                "wvb": wv.astype(NP_BF16),
                "wq8": wq.astype(NP_F8),
                "wk8": wk.astype(NP_F8),
                "wv8": wv.astype(NP_F8),
                "bq": np.ascontiguousarray(bqk[:C][cols]) * W_SCALE,
                "bk": np.ascontiguousarray(bqk[C:][cols]) * W_SCALE,
                "bv": np.ascontiguousarray(bv[cols]),
            }
        )
    return in_maps


def assemble_output(results, B):
    y = np.empty((B, S, C), dtype=np.float32)
    for c in range(N_CORES):
        b, g = divmod(c, 4)
        y[b, :, DCOLS * g : DCOLS * (g + 1)] = (
            np.asarray(results[c]["yT"]).astype(np.float32).T
        )
    return y


def kernel(x, Wqk, bqk, Wv, bv):
    in_maps = make_in_maps(x, Wqk, bqk, Wv, bv)
    with_bqk = bool(np.any(np.asarray(bqk) != 0))
    with_bv = bool(np.any(np.asarray(bv) != 0))
    nc = get_program(with_bqk, with_bv)
    res = run_bass_kernel_spmd(nc, in_maps, core_ids=list(range(N_CORES)))
    return assemble_output(res.results, np.asarray(x).shape[0])


if __name__ == "__main__":
    rng = np.random.default_rng(0)
    x = rng.standard_normal((2, S, C), dtype=np.float32)
    Wqk = rng.standard_normal((C, 2 * C), dtype=np.float32) * 0.02
    bqk = np.zeros((2 * C,), dtype=np.float32)
    Wv = rng.standard_normal((C, C), dtype=np.float32) * 0.02
    bv = np.zeros((C,), dtype=np.float32)
    out = kernel(x, Wqk, bqk, Wv, bv)
    print("kernel output:", out.shape, out.dtype, float(np.abs(out).max()))
